# revision 1
# baseline (speedup 1.0000x reference)
"""GGNN (gated graph NN) message-passing kernel for 8 Trainium2 NeuronCores.

Sharding: edge-type sharding. Core c owns edge-type block c of the adjacency
matrix (columns c*N..(c+1)*N of the [N, 2E*N] adjacency, pre-transposed on the
host) plus the node shard c for the GRU update.

Per step, on core c:
  stage1: t_c = h @ W_prop[c]                      [N, D]   (h^T streamed as lhsT)
  stage2: partial_a_c = A_cT.T @ t_c               [N, D]   (A_cT resident uint8)
  RS:     a_shard = ReduceScatter_add(partial_a)   [N/8, D] (split in 2 halves so
          the first RS overlaps the second half of stage2)
  GRU:    h_shard' = GRU(a_shard, h_shard)         (transposed layout, fp32r mm)
  AG:     h^T' = AllGather(h_shard'^T)             (fp32r)

Each core's node shard is blocks {128c..128c+127, 1024+128c..1024+128c+127}
(the blocks the two half-ReduceScatters deliver to rank c).

Numerics: matmuls in float32r (fp32 with 12-bit mantissa, full PE rate at
free-dim>=256); adjacency stored as uint8 (exact for 0/1) upconverted to fp32r
on DVE; accumulation fp32 in PSUM; elementwise GRU update in fp32.
"""
import sys
if "/opt/trn_rl_repo" not in sys.path:
    sys.path.insert(0, "/opt/trn_rl_repo")

import numpy as np
import ml_dtypes

NC_CORES = 8
N = 2048          # nodes
D = 512           # state dim
ANN = 256         # annotation dim
STEPS = 5
SH = N // NC_CORES   # 256 nodes per shard
KT = D // 128        # 4
MT = N // 128        # 16


def _q12(x):
    """Round fp32 to 12 explicit mantissa bits (fp32r grid), RNE."""
    mant, ex = np.frexp(np.asarray(x, np.float32).astype(np.float64))
    return (np.round(mant * 4096) / 4096 * np.exp2(ex)).astype(np.float32)


def build(repeats=1, ablate=()):
    import concourse.bacc as bacc
    import concourse.mybir as mybir
    import concourse.tile as tile
    from concourse.masks import make_identity

    dt = mybir.dt
    nc = bacc.Bacc()
    at_p = nc.declare_dram_parameter("at", [N, N], dt.uint8, isOutput=False)
    h0t_p = nc.declare_dram_parameter("h0t", [NC_CORES * D, SH], dt.float32r,
                                      isOutput=False)
    h0sr_p = nc.declare_dram_parameter("h0sr", [D, SH], dt.float32r, isOutput=False)
    h0s_p = nc.declare_dram_parameter("h0s", [D, SH], dt.float32, isOutput=False)
    wc_p = nc.declare_dram_parameter("wc", [D, D], dt.float32r, isOutput=False)
    gw_p = nc.declare_dram_parameter("gw", [6, D, D], dt.float32r, isOutput=False)
    bpc_p = nc.declare_dram_parameter("bpc", [1, D], dt.float32, isOutput=False)
    bz_p = nc.declare_dram_parameter("bzc", [D, 1], dt.float32, isOutput=False)
    br_p = nc.declare_dram_parameter("brc", [D, 1], dt.float32, isOutput=False)
    bh_p = nc.declare_dram_parameter("bhc", [D, 1], dt.float32, isOutput=False)
    out_p = nc.declare_dram_parameter("out", [D, SH], dt.float32, isOutput=True)
    RG = [list(range(NC_CORES))]

    from contextlib import ExitStack
    with tile.TileContext(nc) as tc, ExitStack() as stk:
        res = stk.enter_context(tc.tile_pool(name="res", bufs=1))
        p_mm = stk.enter_context(tc.tile_pool(name="pmm", bufs=8, space="PSUM"))
        p_hc = stk.enter_context(tc.tile_pool(name="phc", bufs=6))
        p_t = stk.enter_context(tc.tile_pool(name="pt", bufs=1))
        p_ar = stk.enter_context(tc.tile_pool(name="par", bufs=3))
        p_asb = stk.enter_context(tc.tile_pool(name="pasb", bufs=2))
        p_sm = stk.enter_context(tc.tile_pool(name="psm", bufs=1))
        p_h = stk.enter_context(tc.tile_pool(name="ph", bufs=2))
        dram = stk.enter_context(tc.tile_pool(name="dram", bufs=2, space="DRAM"))

        # ---- setup: constants, weights, adjacency ----
        identity = res.tile([128, 128], dt.float32, tag="identity")
        make_identity(nc, identity[:])
        ones = res.tile([1, 128], dt.float32, tag="ones")
        nc.vector.memset(ones[:], 1.0)
        bpc_t = res.tile([1, D], dt.float32, tag="bpc")
        nc.sync.dma_start(bpc_t[:], bpc_p[:])
        pb = p_mm.tile([128, D], dt.float32, tag="mm")
        nc.tensor.matmul(pb[:], ones[:], bpc_t[:], start=True, stop=True)
        bias_bcast = res.tile([128, D], dt.float32, tag="bias_bcast")
        nc.vector.tensor_copy(bias_bcast[:], pb[:])

        bias_tiles = {}
        for nm, par in (("z", bz_p), ("r", br_p), ("h", bh_p)):
            for f in range(KT):
                bt = res.tile([128, 1], dt.float32, tag=f"b{nm}{f}")
                nc.sync.dma_start(bt[:], par[f * 128:(f + 1) * 128, :])
                bias_tiles[(nm, f)] = bt

        wc_t = []
        for k in range(KT):
            w = res.tile([128, D], dt.float32r, tag=f"wc{k}")
            nc.sync.dma_start(w[:], wc_p[k * 128:(k + 1) * 128, :])
            wc_t.append(w)

        at_t = []
        for m in range(MT):
            a = res.tile([128, N], dt.uint8, tag=f"at{m}")
            nc.sync.dma_start(a[:], at_p[m * 128:(m + 1) * 128, :])
            at_t.append(a)

        # resident GRU weights (fp32r), loaded once
        gw_res = []
        for g in range(6):
            w = res.tile([128, KT, D], dt.float32r, tag=f"gwr{g}")
            nc.scalar.dma_start(w[:], gw_p[g].rearrange("(k p) f -> p k f", p=128))
            gw_res.append(w)

        for rep in range(repeats):
          # step-0 h state
          hsh_prev = []   # h^T shard, fp32r (GRU rhs)
          h32_prev = []   # h^T shard, fp32 (elementwise state)
          for k in range(KT):
            hr = p_h.tile([128, SH], dt.float32r, tag=f"hnr{k}")
            nc.sync.dma_start(hr[:], h0sr_p[k * 128:(k + 1) * 128, :])
            hsh_prev.append(hr)
            h3 = p_h.tile([128, SH], dt.float32, tag=f"h32{k}")
            nc.sync.dma_start(h3[:], h0s_p[k * 128:(k + 1) * 128, :])
            h32_prev.append(h3)

          ag_out_prev = None

          for s in range(STEPS):
             # ---- stage 1: t = h @ W_c  (+ b_c via broadcast add on cast) ----
             # shard layout: core cp owns node blocks {128cp, 1024+128cp}
             t_tiles = [None] * MT
             for mp in range(MT // 2):
                 if "s1" not in ablate:
                     hc = p_hc.tile([128, KT, 2, 128], dt.float32r, tag="hc")
                     blk = (h0t_p if s == 0 else ag_out_prev)[512 * mp:512 * (mp + 1), :]
                     nc.sync.dma_start(
                         hc[:], blk.rearrange("(k p) mj -> p k mj", p=128))
                 for mloc in range(2):
                     m = mp + 8 * mloc
                     pt = p_mm.tile([128, D], dt.float32, tag="mm")
                     if "s1" in ablate:
                         nc.tensor.matmul(pt[:], wc_t[0][:, 0:128], wc_t[1][:],
                                          start=True, stop=True)
                     else:
                         for k in range(KT):
                             nc.tensor.matmul(pt[:], hc[:, k, mloc, :], wc_t[k][:],
                                              start=(k == 0), stop=(k == KT - 1))
                     tm = p_t.tile([128, D], dt.float32r, tag=f"t{m}")
                     nc.vector.tensor_add(tm[:], pt[:], bias_bcast[:])
                     t_tiles[m] = tm

             # ---- stage 2: partial_a = A_cT.T @ t; RS per half (overlapped) ----
             rs_outs = []
             for grp in range(2):
                 rs_in = dram.tile([N // 2, D], dt.float32, tag=f"rs_in{grp}",
                                   name=f"rs_in{grp}")
                 pas = [p_mm.tile([128, D], dt.float32, tag="mm", name=f"pa{grp}_{i}")
                        for i in range(8)]
                 if "s2" in ablate:
                     for i in range(8):
                         nc.tensor.matmul(pas[i][:], t_tiles[0][:, 0:128],
                                          t_tiles[1][:], start=True, stop=True)
                 else:
                  for m in range(MT):
                     ar = p_ar.tile([128, 1024], dt.float32r, tag="ar")
                     nc.vector.tensor_copy(ar[:], at_t[m][:, grp * 1024:(grp + 1) * 1024])
                     for i in range(8):
                         nc.tensor.matmul(pas[i][:], ar[:, i * 128:(i + 1) * 128],
                                          t_tiles[m][:],
                                          start=(m == 0), stop=(m == MT - 1))
                 for i in range(8):
                     n = grp * 8 + i
                     asb = p_asb.tile([128, D], dt.float32, tag="asb")
                     if i % 2 == 0:
                         nc.scalar.copy(asb[:], pas[i][:])
                     else:
                         nc.vector.tensor_copy(asb[:], pas[i][:])
                     eng = nc.sync if i % 2 == 0 else nc.scalar
                     eng.dma_start(rs_in[i * 128:(i + 1) * 128, :], asb[:])
                 # RS of this half: core c receives node block grp*1024 + 128c
                 rs_out = dram.tile([128, D], dt.float32, tag=f"rs_out{grp}",
                                    name=f"rs_out{grp}")
                 if "cc" in ablate or "rs" in ablate:
                     nc.sync.dma_start(rs_out[:], rs_in[0:128, :])
                 else:
                     nc.gpsimd.collective_compute(
                         "ReduceScatter", mybir.AluOpType.add, replica_groups=RG,
                         ins=[rs_in[:]], outs=[rs_out[:]])
                 rs_outs.append(rs_out)

             # ---- transpose a_shard -> aT [D, SH] fp32r ----
             # r2=0 chunks (from RS1) transpose while RS2 is still in flight
             an_tiles = []
             for r2 in range(2):
                 an = p_sm.tile([128, D], dt.float32, tag=f"an{r2}")
                 nc.sync.dma_start(an[:], rs_outs[r2][:])
                 an_tiles.append(an)
             aT = []
             for kb in range(KT):
                 a_kb = p_sm.tile([128, SH], dt.float32r, tag=f"aT{kb}")
                 aT.append(a_kb)
             for r2 in range(2):
                 for kb in range(KT):
                     ptr = p_mm.tile([128, 128], dt.float32, tag="mm")
                     nc.tensor.transpose(ptr[:], an_tiles[r2][:, kb * 128:(kb + 1) * 128],
                                         identity[:])
                     nc.vector.tensor_copy(aT[kb][:, r2 * 128:(r2 + 1) * 128], ptr[:])

             # ---- GRU gates (transposed layout [D, SH]) ----
             def gate_mm(widx, uidx, rhs_u, func, bias_nm, out_dtype=dt.float32):
                 Wq, Uq = gw_res[widx], gw_res[uidx]
                 outs = []
                 for f in range(KT):
                     pg = p_mm.tile([128, SH], dt.float32, tag="mm")
                     if "gru" in ablate:
                         nc.tensor.matmul(pg[:], aT[0][:, 0:128], aT[0][:],
                                          start=True, stop=True)
                         nc.tensor.matmul(pg[:], rhs_u[0][:, 0:128], rhs_u[0][:],
                                          start=False, stop=True)
                         k = None
                     else:
                      for k in range(KT):
                         nc.tensor.matmul(pg[:], Wq[:, k, f * 128:(f + 1) * 128],
                                          aT[k][:], start=(k == 0), stop=False)
                      for k in range(KT):
                         nc.tensor.matmul(pg[:], Uq[:, k, f * 128:(f + 1) * 128],
                                          rhs_u[k][:], start=False, stop=(k == KT - 1))
                     og = p_sm.tile([128, SH], out_dtype, tag=f"g{bias_nm}{f}")
                     nc.scalar.activation(og[:], pg[:], func,
                                          bias=bias_tiles[(bias_nm, f)][:])
                     outs.append(og)
                 return outs

             import concourse.mybir as _mb
             if "gru" in ablate:
                 z_t = gate_mm(0, 1, hsh_prev, _mb.ActivationFunctionType.Sigmoid, "z")
                 r_t = gate_mm(2, 3, hsh_prev, _mb.ActivationFunctionType.Sigmoid, "r")
             else:
                 # z and r gates with both U-halves emitted first: the U-term
                 # matmuls depend only on local h and run while RS2 is in flight
                 pz = [p_mm.tile([128, SH], dt.float32, tag="mm", name=f"pz{f}")
                       for f in range(KT)]
                 pr = [p_mm.tile([128, SH], dt.float32, tag="mm", name=f"pr{f}")
                       for f in range(KT)]
                 for pg_l, uidx in ((pz, 1), (pr, 3)):
                     Uq = gw_res[uidx]
                     for f in range(KT):
                         for k in range(KT):
                             nc.tensor.matmul(pg_l[f][:],
                                              Uq[:, k, f * 128:(f + 1) * 128],
                                              hsh_prev[k][:],
                                              start=(k == 0), stop=False)
                 for pg_l, widx in ((pz, 0), (pr, 2)):
                     Wq = gw_res[widx]
                     for f in range(KT):
                         for k in range(KT):
                             nc.tensor.matmul(pg_l[f][:],
                                              Wq[:, k, f * 128:(f + 1) * 128],
                                              aT[k][:],
                                              start=False, stop=(k == KT - 1))
                 z_t, r_t = [], []
                 for outs, pg_l, nm, fn in (
                         (z_t, pz, "z", _mb.ActivationFunctionType.Sigmoid),
                         (r_t, pr, "r", _mb.ActivationFunctionType.Sigmoid)):
                     for f in range(KT):
                         og = p_sm.tile([128, SH], dt.float32, tag=f"g{nm}{f}",
                                        name=f"g{nm}{f}")
                         nc.scalar.activation(og[:], pg_l[f][:], fn,
                                              bias=bias_tiles[(nm, f)][:])
                         outs.append(og)
             rh = []
             for k in range(KT):
                 rhk = p_sm.tile([128, SH], dt.float32r, tag=f"rh{k}")
                 nc.vector.tensor_mul(rhk[:], r_t[k][:], h32_prev[k][:])
                 rh.append(rhk)
             ht_t = gate_mm(4, 5, rh, _mb.ActivationFunctionType.Tanh, "h")

             # ---- h' = h + z * (ht - h) ----
             hsh_new, h32_new = [], []
             last = (s == STEPS - 1)
             if not last:
                 ag_in = dram.tile([D, SH], dt.float32r, tag="ag_in")
             for k in range(KT):
                 s1 = p_sm.tile([128, SH], dt.float32, tag="gsA")
                 nc.vector.tensor_sub(s1[:], ht_t[k][:], h32_prev[k][:])
                 s2 = p_sm.tile([128, SH], dt.float32, tag="gsB")
                 nc.vector.tensor_mul(s2[:], z_t[k][:], s1[:])
                 h3 = p_h.tile([128, SH], dt.float32, tag=f"h32{k}")
                 nc.vector.tensor_add(h3[:], h32_prev[k][:], s2[:])
                 h32_new.append(h3)
                 if last:
                     nc.sync.dma_start(out_p[k * 128:(k + 1) * 128, :], h3[:])
                 else:
                     hr = p_h.tile([128, SH], dt.float32r, tag=f"hnr{k}")
                     nc.vector.tensor_copy(hr[:], h3[:])
                     hsh_new.append(hr)
                     nc.sync.dma_start(ag_in[k * 128:(k + 1) * 128, :], hr[:])

             if not last:
                 ag_out = dram.tile([NC_CORES * D, SH], dt.float32r, tag="ag_out",
                                    addr_space="Shared")
                 if "cc" in ablate or "ag" in ablate:
                     nc.sync.dma_start(ag_out[0:D, :], ag_in[:])
                 else:
                     nc.gpsimd.collective_compute(
                         "AllGather", mybir.AluOpType.bypass, replica_groups=RG,
                         ins=[ag_in[:]], outs=[ag_out[:]])
                 ag_out_prev = ag_out
                 hsh_prev, h32_prev = hsh_new, h32_new

    nc.finalize()
    return nc


_BUILT = None
TRACE = False
LAST_RESULT = None


_BUILT_R = {}


def _get_built(repeats=1, ablate=()):
    global _BUILT
    key = (repeats, tuple(ablate))
    if key != (1, ()):
        if key not in _BUILT_R:
            _BUILT_R[key] = build(repeats, ablate)
        return _BUILT_R[key]
    if _BUILT is None:
        _BUILT = build()
    return _BUILT


def prepare_in_maps(adjacency, annotations, W_prop, b_prop, Wz, Uz, bz,
                    Wr, Ur, br, Wh, Uh, bh):
    A = np.asarray(adjacency, np.float32)
    ann = np.asarray(annotations, np.float32)
    W_prop = np.asarray(W_prop, np.float32)
    b_prop = np.asarray(b_prop, np.float32)
    gw_all = _q12(np.stack([np.asarray(x, np.float32)
                            for x in (Wz, Uz, Wr, Ur, Wh, Uh)]))
    bz = np.asarray(bz, np.float32).reshape(D, 1)
    br = np.asarray(br, np.float32).reshape(D, 1)
    bh = np.asarray(bh, np.float32).reshape(D, 1)

    h0 = np.zeros((N, D), np.float32)
    h0[:, :ann.shape[1]] = ann
    h0t = np.ascontiguousarray(h0.T)           # [D, N] fp32
    h0t_r = _q12(h0t)
    A_T = np.ascontiguousarray(A.T)            # [2E*N, N]

    # shard layout: core c owns node blocks {128c..128c+127, 1024+128c..+127}
    shard_cols = [np.r_[128 * c:128 * c + 128, 1024 + 128 * c:1024 + 128 * c + 128]
                  for c in range(NC_CORES)]
    h0t_ag = np.ascontiguousarray(np.concatenate(
        [h0t_r[:, shard_cols[c]] for c in range(NC_CORES)], axis=0))

    in_maps = []
    for c in range(NC_CORES):
        in_maps.append({
            "at": np.ascontiguousarray(
                A_T[c * N:(c + 1) * N, :]).astype(np.uint8),
            "h0t": h0t_ag,
            "h0sr": np.ascontiguousarray(h0t_r[:, shard_cols[c]]),
            "h0s": np.ascontiguousarray(h0t[:, shard_cols[c]]),
            "wc": _q12(W_prop[c]),
            "gw": gw_all,
            "bpc": np.ascontiguousarray(b_prop[c].reshape(1, D)),
            "bzc": bz, "brc": br, "bhc": bh,
        })

    return in_maps


def kernel(**inputs):
    from concourse.bass_utils import run_bass_kernel_spmd

    in_maps = prepare_in_maps(
        **{k: inputs[k] for k in ("adjacency", "annotations", "W_prop", "b_prop",
                                  "Wz", "Uz", "bz", "Wr", "Ur", "br",
                                  "Wh", "Uh", "bh")})
    nc = _get_built()
    res = run_bass_kernel_spmd(nc, in_maps, list(range(NC_CORES)), trace=TRACE)
    global LAST_RESULT
    LAST_RESULT = res
    h = np.empty((N, D), np.float32)
    for c in range(NC_CORES):
        sh = res.results[c]["out"].T           # [SH, D] rows in shard order
        h[128 * c:128 * c + 128] = sh[:128]
        h[1024 + 128 * c:1024 + 128 * c + 128] = sh[128:]
    return h



# revision 19
# speedup vs baseline: 1.1921x; 1.1921x over previous
"""GGNN (gated graph NN) message-passing kernel for 8 Trainium2 NeuronCores.

Sharding: edge-type sharding. Core c owns edge-type block c of the adjacency
matrix (columns c*N..(c+1)*N of the [N, 2E*N] adjacency, pre-transposed on the
host) plus the node shard c for the GRU update.

Per step, on core c:
  stage1: t_c = h @ W_prop[c]                      [N, D]   (h^T streamed as lhsT)
  stage2: partial_a_c = A_cT.T @ t_c               [N, D]   (A_cT resident uint8)
  RS:     a_shard = ReduceScatter_add(partial_a)   [N/8, D] (split in 2 halves so
          the first RS overlaps the second half of stage2)
  GRU:    h_shard' = GRU(a_shard, h_shard)         (transposed layout, fp32r mm)
  AG:     h^T' = AllGather(h_shard'^T)             (fp32r)

Each core's node shard is blocks {128c..128c+127, 1024+128c..1024+128c+127}
(the blocks the two half-ReduceScatters deliver to rank c).

Numerics: weight matmul operands in float32r (fp32 with 12-bit mantissa, full
PE rate at free-dim>=256); collective-carried tensors (partial_a via RS, h via
AG) in float16 to halve collective bytes — fp16 moving operands also run at
full PE rate; adjacency stored as uint8 (exact for 0/1) upconverted to fp32r
on DVE; accumulation fp32 in PSUM; elementwise GRU update in fp32.
"""
import sys
if "/opt/trn_rl_repo" not in sys.path:
    sys.path.insert(0, "/opt/trn_rl_repo")

import numpy as np
import ml_dtypes

NC_CORES = 8
N = 2048          # nodes
D = 512           # state dim
ANN = 256         # annotation dim
STEPS = 5
SH = N // NC_CORES   # 256 nodes per shard
KT = D // 128        # 4
MT = N // 128        # 16


def _q12(x):
    """Round fp32 to 12 explicit mantissa bits (fp32r grid), RNE."""
    mant, ex = np.frexp(np.asarray(x, np.float32).astype(np.float64))
    return (np.round(mant * 4096) / 4096 * np.exp2(ex)).astype(np.float32)


def build(repeats=1, ablate=()):
    import concourse.bacc as bacc
    import concourse.mybir as mybir
    import concourse.tile as tile
    from concourse.masks import make_identity

    dt = mybir.dt
    nc = bacc.Bacc()
    at_p = nc.declare_dram_parameter("at", [N, N], dt.uint8, isOutput=False)
    h0t_p = nc.declare_dram_parameter("h0t", [NC_CORES * D, SH], dt.float16,
                                      isOutput=False)
    h0sr_p = nc.declare_dram_parameter("h0sr", [D, SH], dt.float16, isOutput=False)
    h0s_p = nc.declare_dram_parameter("h0s", [D, SH], dt.float32, isOutput=False)
    wc_p = nc.declare_dram_parameter("wc", [D, D], dt.float16, isOutput=False)
    gw_p = nc.declare_dram_parameter("gw", [6, D, D], dt.float16, isOutput=False)
    bpc_p = nc.declare_dram_parameter("bpc", [1, D], dt.float32, isOutput=False)
    bz_p = nc.declare_dram_parameter("bzc", [D, 1], dt.float32, isOutput=False)
    br_p = nc.declare_dram_parameter("brc", [D, 1], dt.float32, isOutput=False)
    bh_p = nc.declare_dram_parameter("bhc", [D, 1], dt.float32, isOutput=False)
    out_p = nc.declare_dram_parameter("out", [D, SH], dt.float32, isOutput=True)
    RG = [list(range(NC_CORES))]

    from contextlib import ExitStack
    with tile.TileContext(nc) as tc, ExitStack() as stk:
        res = stk.enter_context(tc.tile_pool(name="res", bufs=1))
        p_mm = stk.enter_context(tc.tile_pool(name="pmm", bufs=8, space="PSUM"))
        p_hc = stk.enter_context(tc.tile_pool(name="phc", bufs=6))
        p_t = stk.enter_context(tc.tile_pool(name="pt", bufs=1))
        p_ar = stk.enter_context(tc.tile_pool(name="par", bufs=3))
        p_asb = stk.enter_context(tc.tile_pool(name="pasb", bufs=2))
        p_sm = stk.enter_context(tc.tile_pool(name="psm", bufs=1))
        p_h = stk.enter_context(tc.tile_pool(name="ph", bufs=2))
        dram = stk.enter_context(tc.tile_pool(name="dram", bufs=2, space="DRAM"))

        # ---- setup: constants, weights, adjacency ----
        identity = res.tile([128, 128], dt.float32, tag="identity")
        make_identity(nc, identity[:])
        identity16 = res.tile([128, 128], dt.float16, tag="identity16")
        nc.vector.tensor_copy(identity16[:], identity[:])
        ones = res.tile([1, 128], dt.float32, tag="ones")
        nc.vector.memset(ones[:], 1.0)
        bpc_t = res.tile([1, D], dt.float32, tag="bpc")
        nc.sync.dma_start(bpc_t[:], bpc_p[:])
        pb = p_mm.tile([128, D], dt.float32, tag="mm")
        nc.tensor.matmul(pb[:], ones[:], bpc_t[:], start=True, stop=True)
        bias_bcast = res.tile([128, D], dt.float32, tag="bias_bcast")
        nc.vector.tensor_copy(bias_bcast[:], pb[:])

        bias_tiles = {}
        for nm, par in (("z", bz_p), ("r", br_p), ("h", bh_p)):
            for f in range(KT):
                bt = res.tile([128, 1], dt.float32, tag=f"b{nm}{f}")
                nc.sync.dma_start(bt[:], par[f * 128:(f + 1) * 128, :])
                bias_tiles[(nm, f)] = bt

        wc_t = []
        for k in range(KT):
            w = res.tile([128, D], dt.float16, tag=f"wc{k}")
            nc.sync.dma_start(w[:], wc_p[k * 128:(k + 1) * 128, :])
            wc_t.append(w)

        at_t = []
        for m in range(MT):
            a = res.tile([128, N], dt.uint8, tag=f"at{m}")
            nc.sync.dma_start(a[:], at_p[m * 128:(m + 1) * 128, :])
            at_t.append(a)

        # resident GRU weights (fp16), loaded once
        gw_res = []
        for g in range(6):
            w = res.tile([128, KT, D], dt.float16, tag=f"gwr{g}")
            nc.scalar.dma_start(w[:], gw_p[g].rearrange("(k p) f -> p k f", p=128))
            gw_res.append(w)

        for rep in range(repeats):
          # step-0 h state
          hsh_prev = []   # h^T shard, fp32r (GRU rhs)
          h32_prev = []   # h^T shard, fp32 (elementwise state)
          for k in range(KT):
            hr = p_h.tile([128, SH], dt.float16, tag=f"hnr{k}")
            nc.sync.dma_start(hr[:], h0sr_p[k * 128:(k + 1) * 128, :])
            hsh_prev.append(hr)
            h3 = p_h.tile([128, SH], dt.float32, tag=f"h32{k}")
            nc.sync.dma_start(h3[:], h0s_p[k * 128:(k + 1) * 128, :])
            h32_prev.append(h3)

          ag_out_prev = None

          for s in range(STEPS):
             # ---- stage 1: t = h @ W_c  (+ b_c via broadcast add on cast) ----
             # shard layout: core cp owns node blocks {128cp, 1024+128cp}
             t_tiles = [None] * MT
             for mp in range(MT // 2):
                 if "s1" not in ablate:
                     hc = p_hc.tile([128, KT, 2, 128], dt.float16, tag="hc")
                     blk = (h0t_p if s == 0 else ag_out_prev)[512 * mp:512 * (mp + 1), :]
                     nc.sync.dma_start(
                         hc[:], blk.rearrange("(k p) mj -> p k mj", p=128))
                 for mloc in range(2):
                     m = mp + 8 * mloc
                     pt = p_mm.tile([128, D], dt.float32, tag="mm")
                     if "s1" in ablate:
                         nc.tensor.matmul(pt[:], wc_t[0][:, 0:128], wc_t[1][:],
                                          start=True, stop=True)
                     else:
                         for k in range(KT):
                             nc.tensor.matmul(pt[:], hc[:, k, mloc, :], wc_t[k][:],
                                              start=(k == 0), stop=(k == KT - 1))
                     tm = p_t.tile([128, D], dt.float32r, tag=f"t{m}")
                     nc.vector.tensor_add(tm[:], pt[:], bias_bcast[:])
                     t_tiles[m] = tm

             # ---- stage 2: partial_a = A_cT.T @ t; RS per half (overlapped) ----
             rs_outs = []
             for grp in range(2):
                 rs_in = dram.tile([N // 2, D], dt.float16, tag=f"rs_in{grp}",
                                   name=f"rs_in{grp}")
                 pas = [p_mm.tile([128, D], dt.float32, tag="mm", name=f"pa{grp}_{i}")
                        for i in range(8)]
                 if "s2" in ablate:
                     for i in range(8):
                         nc.tensor.matmul(pas[i][:], t_tiles[0][:, 0:128],
                                          t_tiles[1][:], start=True, stop=True)
                 else:
                  for m in range(MT):
                     ar = p_ar.tile([128, 1024], dt.float32r, tag="ar")
                     nc.vector.tensor_copy(ar[:], at_t[m][:, grp * 1024:(grp + 1) * 1024])
                     for i in range(8):
                         nc.tensor.matmul(pas[i][:], ar[:, i * 128:(i + 1) * 128],
                                          t_tiles[m][:],
                                          start=(m == 0), stop=(m == MT - 1))
                 for i in range(8):
                     n = grp * 8 + i
                     asb = p_asb.tile([128, D], dt.float16, tag="asb")
                     if i % 2 == 0:
                         nc.scalar.copy(asb[:], pas[i][:])
                     else:
                         nc.vector.tensor_copy(asb[:], pas[i][:])
                     eng = nc.sync if i % 2 == 0 else nc.scalar
                     eng.dma_start(rs_in[i * 128:(i + 1) * 128, :], asb[:])
                 # RS of this half: core c receives node block grp*1024 + 128c
                 rs_out = dram.tile([128, D], dt.float16, tag=f"rs_out{grp}",
                                    name=f"rs_out{grp}")
                 if "cc" in ablate or "rs" in ablate:
                     nc.sync.dma_start(rs_out[:], rs_in[0:128, :])
                 else:
                     nc.gpsimd.collective_compute(
                         "ReduceScatter", mybir.AluOpType.add, replica_groups=RG,
                         ins=[rs_in[:]], outs=[rs_out[:]])
                 rs_outs.append(rs_out)

             # ---- transpose a_shard -> aT [D, SH] fp32r ----
             # r2=0 chunks (from RS1) transpose while RS2 is still in flight
             an_tiles = []
             for r2 in range(2):
                 an = p_sm.tile([128, D], dt.float16, tag=f"an{r2}")
                 nc.sync.dma_start(an[:], rs_outs[r2][:])
                 an_tiles.append(an)
             aT = []
             for kb in range(KT):
                 a_kb = p_sm.tile([128, SH], dt.float16, tag=f"aT{kb}")
                 aT.append(a_kb)
             for r2 in range(2):
                 for kb in range(KT):
                     ptr = p_mm.tile([128, 128], dt.float16, tag="mm")
                     nc.tensor.transpose(ptr[:], an_tiles[r2][:, kb * 128:(kb + 1) * 128],
                                         identity16[:])
                     nc.vector.tensor_copy(aT[kb][:, r2 * 128:(r2 + 1) * 128], ptr[:])

             # ---- GRU gates (transposed layout [D, SH]) ----
             def gate_mm(widx, uidx, rhs_u, func, bias_nm, out_dtype=dt.float32):
                 Wq, Uq = gw_res[widx], gw_res[uidx]
                 outs = []
                 for f in range(KT):
                     pg = p_mm.tile([128, SH], dt.float32, tag="mm")
                     if "gru" in ablate:
                         nc.tensor.matmul(pg[:], aT[0][:, 0:128], aT[0][:],
                                          start=True, stop=True)
                         nc.tensor.matmul(pg[:], rhs_u[0][:, 0:128], rhs_u[0][:],
                                          start=False, stop=True)
                         k = None
                     else:
                      for k in range(KT):
                         nc.tensor.matmul(pg[:], Wq[:, k, f * 128:(f + 1) * 128],
                                          aT[k][:], start=(k == 0), stop=False)
                      for k in range(KT):
                         nc.tensor.matmul(pg[:], Uq[:, k, f * 128:(f + 1) * 128],
                                          rhs_u[k][:], start=False, stop=(k == KT - 1))
                     og = p_sm.tile([128, SH], out_dtype, tag=f"g{bias_nm}{f}")
                     nc.scalar.activation(og[:], pg[:], func,
                                          bias=bias_tiles[(bias_nm, f)][:])
                     outs.append(og)
                 return outs

             import concourse.mybir as _mb
             if "gru" in ablate:
                 z_t = gate_mm(0, 1, hsh_prev, _mb.ActivationFunctionType.Sigmoid, "z")
                 r_t = gate_mm(2, 3, hsh_prev, _mb.ActivationFunctionType.Sigmoid, "r")
             else:
                 # z and r gates with both U-halves emitted first: the U-term
                 # matmuls depend only on local h and run while RS2 is in flight
                 pz = [p_mm.tile([128, SH], dt.float32, tag="mm", name=f"pz{f}")
                       for f in range(KT)]
                 pr = [p_mm.tile([128, SH], dt.float32, tag="mm", name=f"pr{f}")
                       for f in range(KT)]
                 for pg_l, uidx in ((pz, 1), (pr, 3)):
                     Uq = gw_res[uidx]
                     for f in range(KT):
                         for k in range(KT):
                             nc.tensor.matmul(pg_l[f][:],
                                              Uq[:, k, f * 128:(f + 1) * 128],
                                              hsh_prev[k][:],
                                              start=(k == 0), stop=False)
                 for pg_l, widx in ((pz, 0), (pr, 2)):
                     Wq = gw_res[widx]
                     for f in range(KT):
                         for k in range(KT):
                             nc.tensor.matmul(pg_l[f][:],
                                              Wq[:, k, f * 128:(f + 1) * 128],
                                              aT[k][:],
                                              start=False, stop=(k == KT - 1))
                 z_t, r_t = [], []
                 for outs, pg_l, nm, fn in (
                         (z_t, pz, "z", _mb.ActivationFunctionType.Sigmoid),
                         (r_t, pr, "r", _mb.ActivationFunctionType.Sigmoid)):
                     for f in range(KT):
                         og = p_sm.tile([128, SH], dt.float32, tag=f"g{nm}{f}",
                                        name=f"g{nm}{f}")
                         nc.scalar.activation(og[:], pg_l[f][:], fn,
                                              bias=bias_tiles[(nm, f)][:])
                         outs.append(og)
             rh = []
             for k in range(KT):
                 rhk = p_sm.tile([128, SH], dt.float16, tag=f"rh{k}")
                 nc.vector.tensor_mul(rhk[:], r_t[k][:], h32_prev[k][:])
                 rh.append(rhk)
             ht_t = gate_mm(4, 5, rh, _mb.ActivationFunctionType.Tanh, "h")

             # ---- h' = h + z * (ht - h) ----
             hsh_new, h32_new = [], []
             last = (s == STEPS - 1)
             if not last:
                 ag_in = dram.tile([D, SH], dt.float16, tag="ag_in")
             for k in range(KT):
                 s1 = p_sm.tile([128, SH], dt.float32, tag="gsA")
                 nc.vector.tensor_sub(s1[:], ht_t[k][:], h32_prev[k][:])
                 s2 = p_sm.tile([128, SH], dt.float32, tag="gsB")
                 nc.vector.tensor_mul(s2[:], z_t[k][:], s1[:])
                 h3 = p_h.tile([128, SH], dt.float32, tag=f"h32{k}")
                 nc.vector.tensor_add(h3[:], h32_prev[k][:], s2[:])
                 h32_new.append(h3)
                 if last:
                     nc.sync.dma_start(out_p[k * 128:(k + 1) * 128, :], h3[:])
                 else:
                     hr = p_h.tile([128, SH], dt.float16, tag=f"hnr{k}")
                     nc.vector.tensor_copy(hr[:], h3[:])
                     hsh_new.append(hr)
                     nc.sync.dma_start(ag_in[k * 128:(k + 1) * 128, :], hr[:])

             if not last:
                 ag_out = dram.tile([NC_CORES * D, SH], dt.float16, tag="ag_out",
                                    addr_space="Shared")
                 if "cc" in ablate or "ag" in ablate:
                     nc.sync.dma_start(ag_out[0:D, :], ag_in[:])
                 else:
                     nc.gpsimd.collective_compute(
                         "AllGather", mybir.AluOpType.bypass, replica_groups=RG,
                         ins=[ag_in[:]], outs=[ag_out[:]])
                 ag_out_prev = ag_out
                 hsh_prev, h32_prev = hsh_new, h32_new

    nc.finalize()
    return nc


_BUILT = None
TRACE = False
LAST_RESULT = None


_BUILT_R = {}


def _get_built(repeats=1, ablate=()):
    global _BUILT
    key = (repeats, tuple(ablate))
    if key != (1, ()):
        if key not in _BUILT_R:
            _BUILT_R[key] = build(repeats, ablate)
        return _BUILT_R[key]
    if _BUILT is None:
        _BUILT = build()
    return _BUILT


def prepare_in_maps(adjacency, annotations, W_prop, b_prop, Wz, Uz, bz,
                    Wr, Ur, br, Wh, Uh, bh):
    A = np.asarray(adjacency, np.float32)
    ann = np.asarray(annotations, np.float32)
    W_prop = np.asarray(W_prop, np.float32)
    b_prop = np.asarray(b_prop, np.float32)
    gw_all = np.stack([np.asarray(x, np.float32)
                       for x in (Wz, Uz, Wr, Ur, Wh, Uh)]).astype(np.float16)
    bz = np.asarray(bz, np.float32).reshape(D, 1)
    br = np.asarray(br, np.float32).reshape(D, 1)
    bh = np.asarray(bh, np.float32).reshape(D, 1)

    h0 = np.zeros((N, D), np.float32)
    h0[:, :ann.shape[1]] = ann
    h0t = np.ascontiguousarray(h0.T)           # [D, N] fp32
    h0t_r = h0t.astype(np.float16)
    A_T = np.ascontiguousarray(A.T)            # [2E*N, N]

    # shard layout: core c owns node blocks {128c..128c+127, 1024+128c..+127}
    shard_cols = [np.r_[128 * c:128 * c + 128, 1024 + 128 * c:1024 + 128 * c + 128]
                  for c in range(NC_CORES)]
    h0t_ag = np.ascontiguousarray(np.concatenate(
        [h0t_r[:, shard_cols[c]] for c in range(NC_CORES)], axis=0))

    in_maps = []
    for c in range(NC_CORES):
        in_maps.append({
            "at": np.ascontiguousarray(
                A_T[c * N:(c + 1) * N, :]).astype(np.uint8),
            "h0t": h0t_ag,
            "h0sr": np.ascontiguousarray(h0t_r[:, shard_cols[c]]),
            "h0s": np.ascontiguousarray(h0t[:, shard_cols[c]]),
            "wc": W_prop[c].astype(np.float16),
            "gw": gw_all,
            "bpc": np.ascontiguousarray(b_prop[c].reshape(1, D)),
            "bzc": bz, "brc": br, "bhc": bh,
        })

    return in_maps


def kernel(**inputs):
    from concourse.bass_utils import run_bass_kernel_spmd

    in_maps = prepare_in_maps(
        **{k: inputs[k] for k in ("adjacency", "annotations", "W_prop", "b_prop",
                                  "Wz", "Uz", "bz", "Wr", "Ur", "br",
                                  "Wh", "Uh", "bh")})
    nc = _get_built()
    res = run_bass_kernel_spmd(nc, in_maps, list(range(NC_CORES)), trace=TRACE)
    global LAST_RESULT
    LAST_RESULT = res
    h = np.empty((N, D), np.float32)
    for c in range(NC_CORES):
        sh = res.results[c]["out"].T           # [SH, D] rows in shard order
        h[128 * c:128 * c + 128] = sh[:128]
        h[1024 + 128 * c:1024 + 128 * c + 128] = sh[128:]
    return h



# revision 20
# speedup vs baseline: 1.4171x; 1.1887x over previous
"""GGNN (gated graph NN) message-passing kernel for 8 Trainium2 NeuronCores.

Sharding: edge-type sharding. Core c owns edge-type block c of the adjacency
matrix (columns c*N..(c+1)*N of the [N, 2E*N] adjacency, pre-transposed on the
host) plus node shard c for the GRU update.

Per step, on core c (node shard split in halves A = block {128c},
B = block {1024+128c} — the blocks the two ReduceScatters deliver):
  stage1: t_c = h @ W_prop[c]                       [N, D]
          (emitted as half-A m-tiles then half-B so each half only
           depends on its own AllGather from the previous step)
  stage2: partial_a_c = A_cT.T @ t_c                [N, D]  in 4 sub-groups
          of 4 node-tiles (4 PSUM banks), RS-A issued after sub-group 1,
          RS-B after sub-group 3 (RS-A hides under sub-groups 2-3)
  tail, per half X in {A, B}:
      transpose a_X -> aT_X [D, 128]
      GRU gates on half X (fp16 matmuls, free dim 128)
      h'_X elementwise
      AG-X: AllGather(h'_X^T) -> [8*D, 128]
  Half-A gates run while RS-B is in flight; AG-A runs while half-B gates
  compute; AG-B overlaps next step's stage1 half-A matmuls.

Numerics: stage2 (adjacency GEMM) in float32r (fp32 with 12-bit mantissa,
full PE rate at free-dim>=256); collective-carried tensors (partial_a via RS,
h via AG) and their matmul partners (W_prop, GRU weights) in float16 — fp16
matmuls run at full PE rate at any free size; adjacency stored as uint8
(exact for 0/1) upconverted to fp32r on DVE; accumulation fp32 in PSUM;
elementwise GRU update in fp32.
"""
import sys
if "/opt/trn_rl_repo" not in sys.path:
    sys.path.insert(0, "/opt/trn_rl_repo")

import numpy as np
import ml_dtypes

NC_CORES = 8
N = 2048          # nodes
D = 512           # state dim
ANN = 256         # annotation dim
STEPS = 5
SH = N // NC_CORES   # 256 nodes per shard
HH = SH // 2         # 128 nodes per half-shard
KT = D // 128        # 4
MT = N // 128        # 16


def build(repeats=1, ablate=()):
    import concourse.bacc as bacc
    import concourse.mybir as mybir
    import concourse.tile as tile
    from concourse.masks import make_identity

    dt = mybir.dt
    nc = bacc.Bacc()
    at_p = nc.declare_dram_parameter("at", [N, N], dt.uint8, isOutput=False)
    h0t_p = nc.declare_dram_parameter("h0t", [NC_CORES * D, SH], dt.float16,
                                      isOutput=False)
    h0sr_p = nc.declare_dram_parameter("h0sr", [D, SH], dt.float16, isOutput=False)
    h0s_p = nc.declare_dram_parameter("h0s", [D, SH], dt.float32, isOutput=False)
    wc_p = nc.declare_dram_parameter("wc", [D, D], dt.float16, isOutput=False)
    gw_p = nc.declare_dram_parameter("gw", [6, D, D], dt.float16, isOutput=False)
    bpc_p = nc.declare_dram_parameter("bpc", [1, D], dt.float32, isOutput=False)
    bz_p = nc.declare_dram_parameter("bzc", [D, 1], dt.float32, isOutput=False)
    br_p = nc.declare_dram_parameter("brc", [D, 1], dt.float32, isOutput=False)
    bh_p = nc.declare_dram_parameter("bhc", [D, 1], dt.float32, isOutput=False)
    out_p = nc.declare_dram_parameter("out", [D, SH], dt.float32, isOutput=True)
    RG = [list(range(NC_CORES))]

    from contextlib import ExitStack
    with tile.TileContext(nc) as tc, ExitStack() as stk:
        res = stk.enter_context(tc.tile_pool(name="res", bufs=1))
        # PSUM: bank-granular (8 banks). stage1/stage2 chains use p_mm (5),
        # gate chains + transposes use p_g (3).
        p_mm = stk.enter_context(tc.tile_pool(name="pmm", bufs=5, space="PSUM"))
        p_g = stk.enter_context(tc.tile_pool(name="pg", bufs=3, space="PSUM"))
        p_hc = stk.enter_context(tc.tile_pool(name="phc", bufs=6))
        p_t = stk.enter_context(tc.tile_pool(name="pt", bufs=1))
        p_ar = stk.enter_context(tc.tile_pool(name="par", bufs=3))
        p_asb = stk.enter_context(tc.tile_pool(name="pasb", bufs=4))
        p_sm = stk.enter_context(tc.tile_pool(name="psm", bufs=2))
        p_h = stk.enter_context(tc.tile_pool(name="ph", bufs=2))
        dram = stk.enter_context(tc.tile_pool(name="dram", bufs=2, space="DRAM"))

        # ---- setup: constants, weights, adjacency ----
        identity = res.tile([128, 128], dt.float32, tag="identity")
        make_identity(nc, identity[:])
        identity16 = res.tile([128, 128], dt.float16, tag="identity16")
        nc.vector.tensor_copy(identity16[:], identity[:])
        ones = res.tile([1, 128], dt.float32, tag="ones")
        nc.vector.memset(ones[:], 1.0)
        bpc_t = res.tile([1, D], dt.float32, tag="bpc")
        nc.sync.dma_start(bpc_t[:], bpc_p[:])
        pb = p_mm.tile([128, D], dt.float32, tag="mm")
        nc.tensor.matmul(pb[:], ones[:], bpc_t[:], start=True, stop=True)
        bias_bcast = res.tile([128, D], dt.float32, tag="bias_bcast")
        nc.vector.tensor_copy(bias_bcast[:], pb[:])

        bias_tiles = {}
        for nm, par in (("z", bz_p), ("r", br_p), ("h", bh_p)):
            for f in range(KT):
                bt = res.tile([128, 1], dt.float32, tag=f"b{nm}{f}")
                nc.sync.dma_start(bt[:], par[f * 128:(f + 1) * 128, :])
                bias_tiles[(nm, f)] = bt

        wc_t = []
        for k in range(KT):
            w = res.tile([128, D], dt.float16, tag=f"wc{k}")
            nc.sync.dma_start(w[:], wc_p[k * 128:(k + 1) * 128, :])
            wc_t.append(w)

        at_t = []
        for m in range(MT):
            a = res.tile([128, N], dt.uint8, tag=f"at{m}")
            nc.sync.dma_start(a[:], at_p[m * 128:(m + 1) * 128, :])
            at_t.append(a)

        # resident GRU weights (fp16), loaded once
        gw_res = []
        for g in range(6):
            w = res.tile([128, KT, D], dt.float16, tag=f"gwr{g}")
            nc.scalar.dma_start(w[:], gw_p[g].rearrange("(k p) f -> p k f", p=128))
            gw_res.append(w)

        import concourse.mybir as _mb
        SIG = _mb.ActivationFunctionType.Sigmoid
        TANH = _mb.ActivationFunctionType.Tanh

        for rep in range(repeats):
          # step-0 h state
          hsh_prev = []   # h^T shard, fp16 (GRU U-term rhs)
          h32_prev = []   # h^T shard, fp32 (elementwise state)
          for k in range(KT):
            hr = p_h.tile([128, SH], dt.float16, tag=f"hnr{k}")
            nc.sync.dma_start(hr[:], h0sr_p[k * 128:(k + 1) * 128, :])
            hsh_prev.append(hr)
            h3 = p_h.tile([128, SH], dt.float32, tag=f"h32{k}")
            nc.sync.dma_start(h3[:], h0s_p[k * 128:(k + 1) * 128, :])
            h32_prev.append(h3)

          ag_prev = None   # pair (agA, agB) of [NC*D, HH] fp16

          for s in range(STEPS):
             # ---- stage 1: t = h @ W_c  (+ b_c via broadcast add on cast) ----
             # half-X m-tiles (mloc=X) only need AG-X of the previous step
             t_tiles = [None] * MT
             for mloc in range(2):
                 for mp in range(8):
                     m = mp + 8 * mloc
                     if "s1" in ablate:
                         pt = p_mm.tile([128, D], dt.float32, tag="mm")
                         nc.tensor.matmul(pt[:], wc_t[0][:, 0:128], wc_t[1][:],
                                          start=True, stop=True)
                     else:
                         hc = p_hc.tile([128, KT, 128], dt.float16, tag="hc")
                         if s == 0:
                             blk = h0t_p[512 * mp:512 * (mp + 1),
                                         mloc * HH:(mloc + 1) * HH]
                         else:
                             blk = ag_prev[mloc][512 * mp:512 * (mp + 1), :]
                         nc.sync.dma_start(
                             hc[:], blk.rearrange("(k p) j -> p k j", p=128))
                         pt = p_mm.tile([128, D], dt.float32, tag="mm")
                         for k in range(KT):
                             nc.tensor.matmul(pt[:], hc[:, k, :], wc_t[k][:],
                                              start=(k == 0), stop=(k == KT - 1))
                     tm = p_t.tile([128, D], dt.float32r, tag=f"t{m}")
                     nc.vector.tensor_add(tm[:], pt[:], bias_bcast[:])
                     t_tiles[m] = tm

             # ---- stage 2: partial_a = A_cT.T @ t in 4 sub-groups of 4 ----
             rs_ins = [dram.tile([N // 2, D], dt.float16, tag=f"rs_in{g}",
                                 name=f"rs_in{g}") for g in range(2)]
             rs_outs = []
             for sub in range(4):
                 pas = [p_mm.tile([128, D], dt.float32, tag="mm",
                                  name=f"pa{sub}_{i}") for i in range(4)]
                 if "s2" in ablate:
                     for i in range(4):
                         nc.tensor.matmul(pas[i][:], t_tiles[0][:, 0:128],
                                          t_tiles[1][:], start=True, stop=True)
                 else:
                  for m in range(MT):
                     ar = p_ar.tile([128, 512], dt.float32r, tag="ar")
                     nc.vector.tensor_copy(
                         ar[:], at_t[m][:, sub * 512:(sub + 1) * 512])
                     for i in range(4):
                         nc.tensor.matmul(pas[i][:], ar[:, i * 128:(i + 1) * 128],
                                          t_tiles[m][:],
                                          start=(m == 0), stop=(m == MT - 1))
                 for i in range(4):
                     asb = p_asb.tile([128, D], dt.float16, tag="asb")
                     if i % 2 == 0:
                         nc.scalar.copy(asb[:], pas[i][:])
                     else:
                         nc.vector.tensor_copy(asb[:], pas[i][:])
                     eng = nc.sync if i % 2 == 0 else nc.scalar
                     row = 512 * (sub % 2) + i * 128
                     eng.dma_start(rs_ins[sub // 2][row:row + 128, :], asb[:])
                 if sub % 2 == 1:
                     grp = sub // 2
                     # RS of this half: core c receives node block grp*1024+128c
                     rs_out = dram.tile([HH, D], dt.float16, tag=f"rs_out{grp}",
                                        name=f"rs_out{grp}")
                     if "cc" in ablate or "rs" in ablate:
                         nc.sync.dma_start(rs_out[:], rs_ins[grp][0:HH, :])
                     else:
                         nc.gpsimd.collective_compute(
                             "ReduceScatter", mybir.AluOpType.add,
                             replica_groups=RG,
                             ins=[rs_ins[grp][:]], outs=[rs_out[:]])
                     rs_outs.append(rs_out)

             # ---- per-half tail: transpose, gates, h', AG ----
             last = (s == STEPS - 1)
             hsh_new, h32_new = [], []
             ag_new = [None, None]
             for k in range(KT):
                 if not last:
                     hr = p_h.tile([128, SH], dt.float16, tag=f"hnr{k}",
                                   name=f"hnr{k}")
                     hsh_new.append(hr)
                 h3 = p_h.tile([128, SH], dt.float32, tag=f"h32{k}",
                               name=f"h32{k}")
                 h32_new.append(h3)

             for X in range(2):
                 cs = slice(X * HH, (X + 1) * HH)
                 an = p_sm.tile([128, D], dt.float16, tag=f"an{X}")
                 nc.sync.dma_start(an[:], rs_outs[X][:])
                 aT = []
                 for kb in range(KT):
                     ptr = p_g.tile([128, 128], dt.float16, tag="gg",
                                    name=f"ptr{kb}")
                     nc.tensor.transpose(
                         ptr[:], an[:, kb * 128:(kb + 1) * 128], identity16[:])
                     a_kb = p_sm.tile([128, HH], dt.float16, tag=f"aT{kb}")
                     nc.vector.tensor_copy(a_kb[:], ptr[:])
                     aT.append(a_kb)

                 def gate_mm(widx, uidx, rhs_u, rhs_u_sl, func, bias_nm):
                     Wq, Uq = gw_res[widx], gw_res[uidx]
                     outs = []
                     for f in range(KT):
                         pg = p_g.tile([128, HH], dt.float32, tag="gg",
                                       name=f"g{bias_nm}{f}")
                         if "gru" in ablate:
                             nc.tensor.matmul(pg[:], aT[0][:, 0:128], aT[0][:],
                                              start=True, stop=True)
                         else:
                             for k in range(KT):
                                 nc.tensor.matmul(
                                     pg[:], Uq[:, k, f * 128:(f + 1) * 128],
                                     rhs_u[k][:, rhs_u_sl] if rhs_u_sl else
                                     rhs_u[k][:],
                                     start=(k == 0), stop=False)
                             for k in range(KT):
                                 nc.tensor.matmul(
                                     pg[:], Wq[:, k, f * 128:(f + 1) * 128],
                                     aT[k][:],
                                     start=False, stop=(k == KT - 1))
                         og = p_sm.tile([128, HH], dt.float32,
                                        tag=f"g{bias_nm}{f}")
                         nc.scalar.activation(og[:], pg[:], func,
                                              bias=bias_tiles[(bias_nm, f)][:])
                         outs.append(og)
                     return outs

                 z_t = gate_mm(0, 1, hsh_prev, cs, SIG, "z")
                 r_t = gate_mm(2, 3, hsh_prev, cs, SIG, "r")
                 rh = []
                 for k in range(KT):
                     rhk = p_sm.tile([128, HH], dt.float16, tag=f"rh{k}")
                     nc.vector.tensor_mul(rhk[:], r_t[k][:], h32_prev[k][:, cs])
                     rh.append(rhk)
                 ht_t = gate_mm(4, 5, rh, None, TANH, "h")

                 # h' = h + z * (ht - h) on columns of this half
                 if not last:
                     ag_in = dram.tile([D, HH], dt.float16, tag=f"ag_in{X}",
                                       name=f"ag_in{X}")
                 for k in range(KT):
                     s1 = p_sm.tile([128, HH], dt.float32, tag="gsA")
                     nc.vector.tensor_sub(s1[:], ht_t[k][:], h32_prev[k][:, cs])
                     s2 = p_sm.tile([128, HH], dt.float32, tag="gsB")
                     nc.vector.tensor_mul(s2[:], z_t[k][:], s1[:])
                     nc.vector.tensor_add(h32_new[k][:, cs], h32_prev[k][:, cs],
                                          s2[:])
                     if last:
                         nc.sync.dma_start(out_p[k * 128:(k + 1) * 128, cs],
                                           h32_new[k][:, cs])
                     else:
                         nc.vector.tensor_copy(hsh_new[k][:, cs],
                                               h32_new[k][:, cs])
                         nc.sync.dma_start(ag_in[k * 128:(k + 1) * 128, :],
                                           hsh_new[k][:, cs])

                 if not last:
                     ag_out = dram.tile([NC_CORES * D, HH], dt.float16,
                                        tag=f"ag_out{X}", name=f"ag_out{X}",
                                        addr_space="Shared")
                     if "cc" in ablate or "ag" in ablate:
                         nc.sync.dma_start(ag_out[0:D, :], ag_in[:])
                     else:
                         nc.gpsimd.collective_compute(
                             "AllGather", mybir.AluOpType.bypass,
                             replica_groups=RG,
                             ins=[ag_in[:]], outs=[ag_out[:]])
                     ag_new[X] = ag_out

             if not last:
                 ag_prev = ag_new
                 hsh_prev, h32_prev = hsh_new, h32_new

    nc.finalize()
    return nc


_BUILT = None
TRACE = False
LAST_RESULT = None


_BUILT_R = {}


def _get_built(repeats=1, ablate=()):
    global _BUILT
    key = (repeats, tuple(ablate))
    if key != (1, ()):
        if key not in _BUILT_R:
            _BUILT_R[key] = build(repeats, ablate)
        return _BUILT_R[key]
    if _BUILT is None:
        _BUILT = build()
    return _BUILT


def prepare_in_maps(adjacency, annotations, W_prop, b_prop, Wz, Uz, bz,
                    Wr, Ur, br, Wh, Uh, bh):
    A = np.asarray(adjacency, np.float32)
    ann = np.asarray(annotations, np.float32)
    W_prop = np.asarray(W_prop, np.float32)
    b_prop = np.asarray(b_prop, np.float32)
    gw_all = np.stack([np.asarray(x, np.float32)
                       for x in (Wz, Uz, Wr, Ur, Wh, Uh)]).astype(np.float16)
    bz = np.asarray(bz, np.float32).reshape(D, 1)
    br = np.asarray(br, np.float32).reshape(D, 1)
    bh = np.asarray(bh, np.float32).reshape(D, 1)

    h0 = np.zeros((N, D), np.float32)
    h0[:, :ann.shape[1]] = ann
    h0t = np.ascontiguousarray(h0.T)           # [D, N] fp32
    h0t_r = h0t.astype(np.float16)
    A_T = np.ascontiguousarray(A.T)            # [2E*N, N]

    # shard layout: core c owns node blocks {128c..128c+127, 1024+128c..+127}
    shard_cols = [np.r_[128 * c:128 * c + 128, 1024 + 128 * c:1024 + 128 * c + 128]
                  for c in range(NC_CORES)]
    h0t_ag = np.ascontiguousarray(np.concatenate(
        [h0t_r[:, shard_cols[c]] for c in range(NC_CORES)], axis=0))

    in_maps = []
    for c in range(NC_CORES):
        in_maps.append({
            "at": np.ascontiguousarray(
                A_T[c * N:(c + 1) * N, :]).astype(np.uint8),
            "h0t": h0t_ag,
            "h0sr": np.ascontiguousarray(h0t_r[:, shard_cols[c]]),
            "h0s": np.ascontiguousarray(h0t[:, shard_cols[c]]),
            "wc": W_prop[c].astype(np.float16),
            "gw": gw_all,
            "bpc": np.ascontiguousarray(b_prop[c].reshape(1, D)),
            "bzc": bz, "brc": br, "bhc": bh,
        })

    return in_maps


def kernel(**inputs):
    from concourse.bass_utils import run_bass_kernel_spmd

    in_maps = prepare_in_maps(
        **{k: inputs[k] for k in ("adjacency", "annotations", "W_prop", "b_prop",
                                  "Wz", "Uz", "bz", "Wr", "Ur", "br",
                                  "Wh", "Uh", "bh")})
    nc = _get_built()
    res = run_bass_kernel_spmd(nc, in_maps, list(range(NC_CORES)), trace=TRACE)
    global LAST_RESULT
    LAST_RESULT = res
    h = np.empty((N, D), np.float32)
    for c in range(NC_CORES):
        sh = res.results[c]["out"].T           # [SH, D] rows in shard order
        h[128 * c:128 * c + 128] = sh[:128]
        h[1024 + 128 * c:1024 + 128 * c + 128] = sh[128:]
    return h


# revision 27
# speedup vs baseline: 1.5026x; 1.0603x over previous
"""GGNN (gated graph NN) message-passing kernel for 8 Trainium2 NeuronCores.

Sharding: edge-type sharding. Core c owns edge-type block c of the adjacency
matrix (columns c*N..(c+1)*N of the [N, 2E*N] adjacency, pre-transposed on the
host) plus node shard c for the GRU update.

Per step, on core c (node shard split in halves A = block {128c},
B = block {1024+128c} — the blocks the two ReduceScatters deliver):
  stage1: t_c = h @ W_prop[c]                       [N, D]
          (emitted as half-A m-tiles then half-B so each half only
           depends on its own AllGather from the previous step)
  stage2: partial_a_c = A_cT.T @ t_c                [N, D]  in 4 sub-groups
          of 4 node-tiles (4 PSUM banks), RS-A issued after sub-group 1,
          RS-B after sub-group 3 (RS-A hides under sub-groups 2-3)
  tail, per half X in {A, B}:
      transpose a_X -> aT_X [D, 128]
      GRU gates on half X (fp16 matmuls, free dim 128)
      h'_X elementwise
      AG-X: AllGather(h'_X^T) -> [8*D, 128]
  Half-A gates run while RS-B is in flight; AG-A runs while half-B gates
  compute; AG-B overlaps next step's stage1 half-A matmuls.

Numerics: all matmuls in float16 (full PE rate at any free size; adjacency
0/1 is exact in fp16, weights/states lose ~2^-11 relative); collective
payloads fp16; accumulation fp32 in PSUM; elementwise GRU update in fp32.
"""
import sys
if "/opt/trn_rl_repo" not in sys.path:
    sys.path.insert(0, "/opt/trn_rl_repo")

import numpy as np
import ml_dtypes

NC_CORES = 8
N = 2048          # nodes
D = 512           # state dim
ANN = 256         # annotation dim
STEPS = 5
SH = N // NC_CORES   # 256 nodes per shard
HH = SH // 2         # 128 nodes per half-shard
KT = D // 128        # 4
MT = N // 128        # 16


def build(repeats=1, ablate=()):
    import concourse.bacc as bacc
    import concourse.mybir as mybir
    import concourse.tile as tile
    from concourse.masks import make_identity

    dt = mybir.dt
    nc = bacc.Bacc()
    at_p = nc.declare_dram_parameter("at", [N, N], dt.float16, isOutput=False)
    h0t_p = nc.declare_dram_parameter("h0t", [NC_CORES * D, SH], dt.float16,
                                      isOutput=False)
    h0sr_p = nc.declare_dram_parameter("h0sr", [D, SH], dt.float16, isOutput=False)
    h0s_p = nc.declare_dram_parameter("h0s", [D, SH], dt.float32, isOutput=False)
    wc_p = nc.declare_dram_parameter("wc", [D, D], dt.float16, isOutput=False)
    gw_p = nc.declare_dram_parameter("gw", [6, D, D], dt.float16, isOutput=False)
    bpc_p = nc.declare_dram_parameter("bpc", [1, D], dt.float32, isOutput=False)
    bz_p = nc.declare_dram_parameter("bzc", [D, 1], dt.float32, isOutput=False)
    br_p = nc.declare_dram_parameter("brc", [D, 1], dt.float32, isOutput=False)
    bh_p = nc.declare_dram_parameter("bhc", [D, 1], dt.float32, isOutput=False)
    out_p = nc.declare_dram_parameter("out", [D, SH], dt.float32, isOutput=True)
    RG = [list(range(NC_CORES))]

    from contextlib import ExitStack
    with tile.TileContext(nc) as tc, ExitStack() as stk:
        res = stk.enter_context(tc.tile_pool(name="res", bufs=1))
        # PSUM: bank-granular (8 banks). stage1/stage2 chains use p_mm (5),
        # gate chains + transposes use p_g (3).
        p_mm = stk.enter_context(tc.tile_pool(name="pmm", bufs=5, space="PSUM"))
        p_g = stk.enter_context(tc.tile_pool(name="pg", bufs=3, space="PSUM"))
        p_hc = stk.enter_context(tc.tile_pool(name="phc", bufs=6))
        p_t = stk.enter_context(tc.tile_pool(name="pt", bufs=1))
        p_asb = stk.enter_context(tc.tile_pool(name="pasb", bufs=4))
        p_sm = stk.enter_context(tc.tile_pool(name="psm", bufs=2))
        p_h = stk.enter_context(tc.tile_pool(name="ph", bufs=2))
        dram = stk.enter_context(tc.tile_pool(name="dram", bufs=2, space="DRAM"))

        # ---- setup: constants, weights, adjacency ----
        identity = res.tile([128, 128], dt.float32, tag="identity")
        make_identity(nc, identity[:])
        identity16 = res.tile([128, 128], dt.float16, tag="identity16")
        nc.vector.tensor_copy(identity16[:], identity[:])
        ones = res.tile([1, 128], dt.float32, tag="ones")
        nc.vector.memset(ones[:], 1.0)
        bpc_t = res.tile([1, D], dt.float32, tag="bpc")
        nc.sync.dma_start(bpc_t[:], bpc_p[:])
        pb = p_mm.tile([128, D], dt.float32, tag="mm")
        nc.tensor.matmul(pb[:], ones[:], bpc_t[:], start=True, stop=True)
        bias_bcast = res.tile([128, D], dt.float32, tag="bias_bcast")
        nc.vector.tensor_copy(bias_bcast[:], pb[:])

        bias_tiles = {}
        for nm, par in (("z", bz_p), ("r", br_p), ("h", bh_p)):
            for f in range(KT):
                bt = res.tile([128, 1], dt.float32, tag=f"b{nm}{f}")
                nc.sync.dma_start(bt[:], par[f * 128:(f + 1) * 128, :])
                bias_tiles[(nm, f)] = bt

        wc_t = []
        for k in range(KT):
            w = res.tile([128, D], dt.float16, tag=f"wc{k}")
            nc.sync.dma_start(w[:], wc_p[k * 128:(k + 1) * 128, :])
            wc_t.append(w)

        at_t = []
        for m in range(MT):
            a = res.tile([128, N], dt.float16, tag=f"at{m}")
            nc.sync.dma_start(a[:], at_p[m * 128:(m + 1) * 128, :])
            at_t.append(a)

        # resident GRU weights (fp16), loaded once
        gw_res = []
        for g in range(6):
            w = res.tile([128, KT, D], dt.float16, tag=f"gwr{g}")
            nc.scalar.dma_start(w[:], gw_p[g].rearrange("(k p) f -> p k f", p=128))
            gw_res.append(w)

        import concourse.mybir as _mb
        SIG = _mb.ActivationFunctionType.Sigmoid
        TANH = _mb.ActivationFunctionType.Tanh

        for rep in range(repeats):
          # step-0 h state
          hsh_prev = []   # h^T shard, fp16 (GRU U-term rhs)
          h32_prev = []   # h^T shard, fp32 (elementwise state)
          for k in range(KT):
            hr = p_h.tile([128, SH], dt.float16, tag=f"hnr{k}")
            nc.sync.dma_start(hr[:], h0sr_p[k * 128:(k + 1) * 128, :])
            hsh_prev.append(hr)
            h3 = p_h.tile([128, SH], dt.float32, tag=f"h32{k}")
            nc.sync.dma_start(h3[:], h0s_p[k * 128:(k + 1) * 128, :])
            h32_prev.append(h3)

          ag_prev = None   # pair (agA, agB) of [NC*D, HH] fp16

          for s in range(STEPS):
             # ---- stage 1: t = h @ W_c  (+ b_c via broadcast add on cast) ----
             # half-X m-tiles (mloc=X) only need AG-X of the previous step
             t_tiles = [None] * MT
             for mloc in range(2):
                 for mp in range(8):
                     m = mp + 8 * mloc
                     if "s1" in ablate:
                         pt = p_mm.tile([128, D], dt.float32, tag="mm")
                         nc.tensor.matmul(pt[:], wc_t[0][:, 0:128], wc_t[1][:],
                                          start=True, stop=True)
                     else:
                         hc = p_hc.tile([128, KT, 128], dt.float16, tag="hc")
                         if s == 0:
                             blk = h0t_p[512 * mp:512 * (mp + 1),
                                         mloc * HH:(mloc + 1) * HH]
                         else:
                             blk = ag_prev[mloc][512 * mp:512 * (mp + 1), :]
                         nc.sync.dma_start(
                             hc[:], blk.rearrange("(k p) j -> p k j", p=128))
                         pt = p_mm.tile([128, D], dt.float32, tag="mm")
                         for k in range(KT):
                             nc.tensor.matmul(pt[:], hc[:, k, :], wc_t[k][:],
                                              start=(k == 0), stop=(k == KT - 1))
                     tm = p_t.tile([128, D], dt.float16, tag=f"t{m}")
                     nc.vector.tensor_add(tm[:], pt[:], bias_bcast[:])
                     t_tiles[m] = tm

             # ---- stage 2: partial_a = A_cT.T @ t in 4 sub-groups of 4 ----
             rs_ins = [dram.tile([N // 2, D], dt.float16, tag=f"rs_in{g}",
                                 name=f"rs_in{g}") for g in range(2)]
             rs_outs = []
             for sub in range(4):
                 pas = [p_mm.tile([128, D], dt.float32, tag="mm",
                                  name=f"pa{sub}_{i}") for i in range(4)]
                 if "s2" in ablate:
                     for i in range(4):
                         nc.tensor.matmul(pas[i][:], t_tiles[0][:, 0:128],
                                          t_tiles[1][:], start=True, stop=True)
                 else:
                  for m in range(MT):
                     for i in range(4):
                         col = sub * 512 + i * 128
                         nc.tensor.matmul(pas[i][:],
                                          at_t[m][:, col:col + 128],
                                          t_tiles[m][:],
                                          start=(m == 0), stop=(m == MT - 1))
                 for i in range(4):
                     asb = p_asb.tile([128, D], dt.float16, tag="asb")
                     if i % 2 == 0:
                         nc.scalar.copy(asb[:], pas[i][:])
                     else:
                         nc.vector.tensor_copy(asb[:], pas[i][:])
                     eng = nc.sync if i % 2 == 0 else nc.scalar
                     row = 512 * (sub % 2) + i * 128
                     eng.dma_start(rs_ins[sub // 2][row:row + 128, :], asb[:])
                 if sub % 2 == 1:
                     grp = sub // 2
                     # RS of this half: core c receives node block grp*1024+128c
                     rs_out = dram.tile([HH, D], dt.float16, tag=f"rs_out{grp}",
                                        name=f"rs_out{grp}")
                     if "cc" in ablate or "rs" in ablate:
                         nc.sync.dma_start(rs_out[:], rs_ins[grp][0:HH, :])
                     else:
                         nc.gpsimd.collective_compute(
                             "ReduceScatter", mybir.AluOpType.add,
                             replica_groups=RG,
                             ins=[rs_ins[grp][:]], outs=[rs_out[:]])
                     rs_outs.append(rs_out)

             # ---- per-half tail: transpose, gates, h', AG ----
             last = (s == STEPS - 1)
             hsh_new, h32_new = [], []
             ag_new = [None, None]
             for k in range(KT):
                 if not last:
                     hr = p_h.tile([128, SH], dt.float16, tag=f"hnr{k}",
                                   name=f"hnr{k}")
                     hsh_new.append(hr)
                 h3 = p_h.tile([128, SH], dt.float32, tag=f"h32{k}",
                               name=f"h32{k}")
                 h32_new.append(h3)

             for X in range(2):
                 cs = slice(X * HH, (X + 1) * HH)
                 an = p_sm.tile([128, D], dt.float16, tag=f"an{X}")
                 nc.sync.dma_start(an[:], rs_outs[X][:])
                 aT = []
                 for kb in range(KT):
                     ptr = p_g.tile([128, 128], dt.float16, tag="gg",
                                    name=f"ptr{kb}")
                     nc.tensor.transpose(
                         ptr[:], an[:, kb * 128:(kb + 1) * 128], identity16[:])
                     a_kb = p_sm.tile([128, HH], dt.float16, tag=f"aT{kb}")
                     nc.vector.tensor_copy(a_kb[:], ptr[:])
                     aT.append(a_kb)

                 def gate_mm(widx, uidx, rhs_u, rhs_u_sl, func, bias_nm):
                     Wq, Uq = gw_res[widx], gw_res[uidx]
                     outs = []
                     for f in range(KT):
                         pg = p_g.tile([128, HH], dt.float32, tag="gg",
                                       name=f"g{bias_nm}{f}")
                         if "gru" in ablate:
                             nc.tensor.matmul(pg[:], aT[0][:, 0:128], aT[0][:],
                                              start=True, stop=True)
                         else:
                             for k in range(KT):
                                 nc.tensor.matmul(
                                     pg[:], Uq[:, k, f * 128:(f + 1) * 128],
                                     rhs_u[k][:, rhs_u_sl] if rhs_u_sl else
                                     rhs_u[k][:],
                                     start=(k == 0), stop=False)
                             for k in range(KT):
                                 nc.tensor.matmul(
                                     pg[:], Wq[:, k, f * 128:(f + 1) * 128],
                                     aT[k][:],
                                     start=False, stop=(k == KT - 1))
                         og = p_sm.tile([128, HH], dt.float32,
                                        tag=f"g{bias_nm}{f}")
                         nc.scalar.activation(og[:], pg[:], func,
                                              bias=bias_tiles[(bias_nm, f)][:])
                         outs.append(og)
                     return outs

                 z_t = gate_mm(0, 1, hsh_prev, cs, SIG, "z")
                 r_t = gate_mm(2, 3, hsh_prev, cs, SIG, "r")
                 rh = []
                 for k in range(KT):
                     rhk = p_sm.tile([128, HH], dt.float16, tag=f"rh{k}")
                     nc.vector.tensor_mul(rhk[:], r_t[k][:], h32_prev[k][:, cs])
                     rh.append(rhk)
                 ht_t = gate_mm(4, 5, rh, None, TANH, "h")

                 # h' = h + z * (ht - h) on columns of this half
                 if not last:
                     ag_in = dram.tile([D, HH], dt.float16, tag=f"ag_in{X}",
                                       name=f"ag_in{X}")
                 for k in range(KT):
                     s1 = p_sm.tile([128, HH], dt.float32, tag="gsA")
                     nc.vector.tensor_sub(s1[:], ht_t[k][:], h32_prev[k][:, cs])
                     s2 = p_sm.tile([128, HH], dt.float32, tag="gsB")
                     nc.vector.tensor_mul(s2[:], z_t[k][:], s1[:])
                     nc.vector.tensor_add(h32_new[k][:, cs], h32_prev[k][:, cs],
                                          s2[:])
                     if last:
                         nc.sync.dma_start(out_p[k * 128:(k + 1) * 128, cs],
                                           h32_new[k][:, cs])
                     else:
                         nc.vector.tensor_copy(hsh_new[k][:, cs],
                                               h32_new[k][:, cs])
                         nc.sync.dma_start(ag_in[k * 128:(k + 1) * 128, :],
                                           hsh_new[k][:, cs])

                 if not last:
                     ag_out = dram.tile([NC_CORES * D, HH], dt.float16,
                                        tag=f"ag_out{X}", name=f"ag_out{X}",
                                        addr_space="Shared")
                     if "cc" in ablate or "ag" in ablate:
                         nc.sync.dma_start(ag_out[0:D, :], ag_in[:])
                     else:
                         nc.gpsimd.collective_compute(
                             "AllGather", mybir.AluOpType.bypass,
                             replica_groups=RG,
                             ins=[ag_in[:]], outs=[ag_out[:]])
                     ag_new[X] = ag_out

             if not last:
                 ag_prev = ag_new
                 hsh_prev, h32_prev = hsh_new, h32_new

    nc.finalize()
    return nc


_BUILT = None
TRACE = False
LAST_RESULT = None


_BUILT_R = {}


def _get_built(repeats=1, ablate=()):
    global _BUILT
    key = (repeats, tuple(ablate))
    if key != (1, ()):
        if key not in _BUILT_R:
            _BUILT_R[key] = build(repeats, ablate)
        return _BUILT_R[key]
    if _BUILT is None:
        _BUILT = build()
    return _BUILT


def prepare_in_maps(adjacency, annotations, W_prop, b_prop, Wz, Uz, bz,
                    Wr, Ur, br, Wh, Uh, bh):
    A = np.asarray(adjacency, np.float32)
    ann = np.asarray(annotations, np.float32)
    W_prop = np.asarray(W_prop, np.float32)
    b_prop = np.asarray(b_prop, np.float32)
    gw_all = np.stack([np.asarray(x, np.float32)
                       for x in (Wz, Uz, Wr, Ur, Wh, Uh)]).astype(np.float16)
    bz = np.asarray(bz, np.float32).reshape(D, 1)
    br = np.asarray(br, np.float32).reshape(D, 1)
    bh = np.asarray(bh, np.float32).reshape(D, 1)

    h0 = np.zeros((N, D), np.float32)
    h0[:, :ann.shape[1]] = ann
    h0t = np.ascontiguousarray(h0.T)           # [D, N] fp32
    h0t_r = h0t.astype(np.float16)
    A_T = np.ascontiguousarray(A.T)            # [2E*N, N]

    # shard layout: core c owns node blocks {128c..128c+127, 1024+128c..+127}
    shard_cols = [np.r_[128 * c:128 * c + 128, 1024 + 128 * c:1024 + 128 * c + 128]
                  for c in range(NC_CORES)]
    h0t_ag = np.ascontiguousarray(np.concatenate(
        [h0t_r[:, shard_cols[c]] for c in range(NC_CORES)], axis=0))

    in_maps = []
    for c in range(NC_CORES):
        in_maps.append({
            "at": np.ascontiguousarray(
                A_T[c * N:(c + 1) * N, :]).astype(np.float16),
            "h0t": h0t_ag,
            "h0sr": np.ascontiguousarray(h0t_r[:, shard_cols[c]]),
            "h0s": np.ascontiguousarray(h0t[:, shard_cols[c]]),
            "wc": W_prop[c].astype(np.float16),
            "gw": gw_all,
            "bpc": np.ascontiguousarray(b_prop[c].reshape(1, D)),
            "bzc": bz, "brc": br, "bhc": bh,
        })

    return in_maps


def kernel(**inputs):
    from concourse.bass_utils import run_bass_kernel_spmd

    in_maps = prepare_in_maps(
        **{k: inputs[k] for k in ("adjacency", "annotations", "W_prop", "b_prop",
                                  "Wz", "Uz", "bz", "Wr", "Ur", "br",
                                  "Wh", "Uh", "bh")})
    nc = _get_built()
    res = run_bass_kernel_spmd(nc, in_maps, list(range(NC_CORES)), trace=TRACE)
    global LAST_RESULT
    LAST_RESULT = res
    h = np.empty((N, D), np.float32)
    for c in range(NC_CORES):
        sh = res.results[c]["out"].T           # [SH, D] rows in shard order
        h[128 * c:128 * c + 128] = sh[:128]
        h[1024 + 128 * c:1024 + 128 * c + 128] = sh[128:]
    return h


# revision 31
# speedup vs baseline: 1.7276x; 1.1498x over previous
"""GGNN (gated graph NN) message-passing kernel for 8 Trainium2 NeuronCores.

Sharding: edge-type sharding. Core c owns edge-type block c of the adjacency
matrix (columns c*N..(c+1)*N of the [N, 2E*N] adjacency, pre-transposed on the
host) plus node shard c for the GRU update.

Per step, on core c (node shard split in halves A = block {128c},
B = block {1024+128c} — the blocks the two ReduceScatters deliver):
  stage1: t_c = h @ W_prop[c]                       [N, D]
          (emitted as half-A m-tiles then half-B so each half only
           depends on its own AllGather from the previous step)
  stage2: partial_a_c = A_cT.T @ t_c                [N, D]  in 4 sub-groups
          of 4 node-tiles (4 PSUM banks), RS-A issued after sub-group 1,
          RS-B after sub-group 3 (RS-A hides under sub-groups 2-3)
  tail, per half X in {A, B}:
      transpose a_X -> aT_X [D, 128]
      GRU gates on half X (fp16 matmuls, free dim 128)
      h'_X elementwise
      AG-X: AllGather(h'_X^T) -> [8*D, 128]
  Half-A gates run while RS-B is in flight; AG-A runs while half-B gates
  compute; AG-B overlaps next step's stage1 half-A matmuls.

Numerics: all matmuls in float16 (full PE rate at any free size; adjacency
0/1 is exact in fp16, weights/states lose ~2^-11 relative); collective
payloads fp16; accumulation fp32 in PSUM; elementwise GRU update in fp32.
"""
import sys
if "/opt/trn_rl_repo" not in sys.path:
    sys.path.insert(0, "/opt/trn_rl_repo")

import numpy as np
import ml_dtypes

NC_CORES = 8
N = 2048          # nodes
D = 512           # state dim
ANN = 256         # annotation dim
STEPS = 5
SH = N // NC_CORES   # 256 nodes per shard
HH = SH // 2         # 128 nodes per half-shard
KT = D // 128        # 4
MT = N // 128        # 16


def build(repeats=1, ablate=()):
    import concourse.bacc as bacc
    import concourse.mybir as mybir
    import concourse.tile as tile
    from concourse.masks import make_identity

    dt = mybir.dt
    nc = bacc.Bacc()
    at_p = nc.declare_dram_parameter("at", [N, N], dt.float16, isOutput=False)
    h0t_p = nc.declare_dram_parameter("h0t", [NC_CORES * D, SH], dt.float16,
                                      isOutput=False)
    h0sr_p = nc.declare_dram_parameter("h0sr", [D, SH], dt.float16, isOutput=False)
    h0s_p = nc.declare_dram_parameter("h0s", [D, SH], dt.float32, isOutput=False)
    wc_p = nc.declare_dram_parameter("wc", [D, D], dt.float16, isOutput=False)
    gw_p = nc.declare_dram_parameter("gw", [6, D, D], dt.float16, isOutput=False)
    bpc_p = nc.declare_dram_parameter("bpc", [1, D], dt.float32, isOutput=False)
    bz_p = nc.declare_dram_parameter("bzc", [D, 1], dt.float32, isOutput=False)
    br_p = nc.declare_dram_parameter("brc", [D, 1], dt.float32, isOutput=False)
    bh_p = nc.declare_dram_parameter("bhc", [D, 1], dt.float32, isOutput=False)
    out_p = nc.declare_dram_parameter("out", [D, SH], dt.float32, isOutput=True)
    RG = [list(range(NC_CORES))]

    from contextlib import ExitStack
    with tile.TileContext(nc) as tc, ExitStack() as stk:
        res = stk.enter_context(tc.tile_pool(name="res", bufs=1))
        # PSUM: bank-granular (8 banks). stage1/stage2 chains use p_mm (5),
        # gate chains + transposes use p_g (3).
        p_mm = stk.enter_context(tc.tile_pool(name="pmm", bufs=5, space="PSUM"))
        p_g = stk.enter_context(tc.tile_pool(name="pg", bufs=3, space="PSUM"))
        p_hc = stk.enter_context(tc.tile_pool(name="phc", bufs=6))
        p_t = stk.enter_context(tc.tile_pool(name="pt", bufs=1))
        p_pp = stk.enter_context(tc.tile_pool(name="ppp", bufs=1))
        p_asb = stk.enter_context(tc.tile_pool(name="pasb", bufs=4))
        p_sm = stk.enter_context(tc.tile_pool(name="psm", bufs=2))
        p_h = stk.enter_context(tc.tile_pool(name="ph", bufs=2))
        dram = stk.enter_context(tc.tile_pool(name="dram", bufs=2, space="DRAM"))

        # ---- setup: constants, weights, adjacency ----
        identity = res.tile([128, 128], dt.float32, tag="identity")
        make_identity(nc, identity[:])
        identity16 = res.tile([128, 128], dt.float16, tag="identity16")
        nc.vector.tensor_copy(identity16[:], identity[:])
        ones = res.tile([1, 128], dt.float32, tag="ones")
        nc.vector.memset(ones[:], 1.0)
        bpc_t = res.tile([1, D], dt.float32, tag="bpc")
        nc.sync.dma_start(bpc_t[:], bpc_p[:])
        pb = p_mm.tile([128, D], dt.float32, tag="mm")
        nc.tensor.matmul(pb[:], ones[:], bpc_t[:], start=True, stop=True)
        bias_bcast = res.tile([128, D], dt.float32, tag="bias_bcast")
        nc.vector.tensor_copy(bias_bcast[:], pb[:])

        bias_tiles = {}
        for nm, par in (("z", bz_p), ("r", br_p), ("h", bh_p)):
            for f in range(KT):
                bt = res.tile([128, 1], dt.float32, tag=f"b{nm}{f}")
                nc.sync.dma_start(bt[:], par[f * 128:(f + 1) * 128, :])
                bias_tiles[(nm, f)] = bt

        wc_t = []
        for k in range(KT):
            w = res.tile([128, D], dt.float16, tag=f"wc{k}")
            nc.sync.dma_start(w[:], wc_p[k * 128:(k + 1) * 128, :])
            wc_t.append(w)

        at_t = []
        for m in range(MT):
            a = res.tile([128, N], dt.float16, tag=f"at{m}")
            nc.sync.dma_start(a[:], at_p[m * 128:(m + 1) * 128, :])
            at_t.append(a)

        # resident GRU weights (fp16), loaded once
        gw_res = []
        for g in range(6):
            w = res.tile([128, KT, D], dt.float16, tag=f"gwr{g}")
            nc.scalar.dma_start(w[:], gw_p[g].rearrange("(k p) f -> p k f", p=128))
            gw_res.append(w)

        import concourse.mybir as _mb
        SIG = _mb.ActivationFunctionType.Sigmoid
        TANH = _mb.ActivationFunctionType.Tanh

        for rep in range(repeats):
          # step-0 h state
          hsh_prev = []   # h^T shard, fp16 (GRU U-term rhs)
          h32_prev = []   # h^T shard, fp32 (elementwise state)
          for k in range(KT):
            hr = p_h.tile([128, SH], dt.float16, tag=f"hnr{k}")
            nc.sync.dma_start(hr[:], h0sr_p[k * 128:(k + 1) * 128, :])
            hsh_prev.append(hr)
            h3 = p_h.tile([128, SH], dt.float32, tag=f"h32{k}")
            nc.sync.dma_start(h3[:], h0s_p[k * 128:(k + 1) * 128, :])
            h32_prev.append(h3)

          ag_prev = None   # pair (agA, agB) of [NC*D, HH] fp16

          for s in range(STEPS):
             # ---- stage 1 + stage 2, software-pipelined ----
             # stage1 half-X m-tiles (mloc=X) only need AG-X of the previous
             # step. stage2 is split into two contraction phases: phase1
             # (m=0..7, needs only stage1-A) runs between stage1-A and
             # stage1-B so it overlaps the in-flight AG-B; phase2 (m=8..15)
             # adds the SBUF-staged phase-1 partials on the way out. This
             # moves RS-A's inputs ~15us earlier.
             t_tiles = [None] * MT

             def stage1(mloc):
                 for mp in range(8):
                     m = mp + 8 * mloc
                     if "s1" in ablate:
                         pt = p_mm.tile([128, D], dt.float32, tag="mm",
                                        name="pt")
                         nc.tensor.matmul(pt[:], wc_t[0][:, 0:128], wc_t[1][:],
                                          start=True, stop=True)
                     else:
                         hc = p_hc.tile([128, KT, 128], dt.float16, tag="hc",
                                        name="hc")
                         if s == 0:
                             blk = h0t_p[512 * mp:512 * (mp + 1),
                                         mloc * HH:(mloc + 1) * HH]
                         else:
                             blk = ag_prev[mloc][512 * mp:512 * (mp + 1), :]
                         nc.sync.dma_start(
                             hc[:], blk.rearrange("(k p) j -> p k j", p=128))
                         pt = p_mm.tile([128, D], dt.float32, tag="mm",
                                        name="pt")
                         for k in range(KT):
                             nc.tensor.matmul(pt[:], hc[:, k, :], wc_t[k][:],
                                              start=(k == 0), stop=(k == KT - 1))
                     tm = p_t.tile([128, D], dt.float16, tag=f"t{m}", name="tm")
                     nc.vector.tensor_add(tm[:], pt[:], bias_bcast[:])
                     t_tiles[m] = tm

             rs_ins = [dram.tile([N // 2, D], dt.float16, tag=f"rs_in{g}",
                                 name=f"rs_in{g}") for g in range(2)]
             rs_outs = []
             pp = {}

             def s2_phase1(sub):
                 if "s2" in ablate:
                     return
                 pas = [p_mm.tile([128, D], dt.float32, tag="mm",
                                  name=f"pa1_{sub}_{i}") for i in range(4)]
                 for m in range(8):
                     for i in range(4):
                         col = sub * 512 + i * 128
                         nc.tensor.matmul(pas[i][:],
                                          at_t[m][:, col:col + 128],
                                          t_tiles[m][:],
                                          start=(m == 0), stop=(m == 7))
                 for i in range(4):
                     ppt = p_pp.tile([128, D], dt.float32, tag=f"pp{sub}{i}",
                                     name=f"pp{sub}{i}")
                     if i % 2 == 0:
                         nc.scalar.copy(ppt[:], pas[i][:])
                     else:
                         nc.vector.tensor_copy(ppt[:], pas[i][:])
                     pp[(sub, i)] = ppt

             def s2_phase2(sub):
                 pas = [p_mm.tile([128, D], dt.float32, tag="mm",
                                  name=f"pa2_{sub}_{i}") for i in range(4)]
                 if "s2" in ablate:
                     for i in range(4):
                         nc.tensor.matmul(pas[i][:], t_tiles[0][:, 0:128],
                                          t_tiles[1][:], start=True, stop=True)
                 else:
                  for m in range(8, MT):
                     for i in range(4):
                         col = sub * 512 + i * 128
                         nc.tensor.matmul(pas[i][:],
                                          at_t[m][:, col:col + 128],
                                          t_tiles[m][:],
                                          start=(m == 8), stop=(m == MT - 1))
                 for i in range(4):
                     asb = p_asb.tile([128, D], dt.float16, tag="asb",
                                      name="asb")
                     if "s2" in ablate:
                         nc.vector.tensor_copy(asb[:], pas[i][:])
                     else:
                         nc.vector.tensor_add(asb[:], pas[i][:],
                                              pp[(sub, i)][:])
                     eng = nc.sync if i % 2 == 0 else nc.scalar
                     row = 512 * (sub % 2) + i * 128
                     eng.dma_start(rs_ins[sub // 2][row:row + 128, :], asb[:])
                 if sub % 2 == 1:
                     grp = sub // 2
                     # RS of this half: core c receives node block grp*1024+128c
                     rs_out = dram.tile([HH, D], dt.float16, tag=f"rs_out{grp}",
                                        name=f"rs_out{grp}")
                     if "cc" in ablate or "rs" in ablate:
                         nc.sync.dma_start(rs_out[:], rs_ins[grp][0:HH, :])
                     else:
                         nc.gpsimd.collective_compute(
                             "ReduceScatter", mybir.AluOpType.add,
                             replica_groups=RG,
                             ins=[rs_ins[grp][:]], outs=[rs_out[:]])
                     rs_outs.append(rs_out)

             stage1(0)
             s2_phase1(0)
             s2_phase1(1)
             stage1(1)
             s2_phase2(0)
             s2_phase2(1)      # issues RS-A
             s2_phase1(2)
             s2_phase1(3)
             s2_phase2(2)
             s2_phase2(3)      # issues RS-B

             # ---- per-half tail: transpose, gates, h', AG ----
             last = (s == STEPS - 1)
             hsh_new, h32_new = [], []
             ag_new = [None, None]
             for k in range(KT):
                 if not last:
                     hr = p_h.tile([128, SH], dt.float16, tag=f"hnr{k}",
                                   name=f"hnr{k}")
                     hsh_new.append(hr)
                 h3 = p_h.tile([128, SH], dt.float32, tag=f"h32{k}",
                               name=f"h32{k}")
                 h32_new.append(h3)

             for X in range(2):
                 cs = slice(X * HH, (X + 1) * HH)
                 an = p_sm.tile([128, D], dt.float16, tag=f"an{X}")
                 nc.sync.dma_start(an[:], rs_outs[X][:])
                 aT = []
                 for kb in range(KT):
                     ptr = p_g.tile([128, 128], dt.float16, tag="gg",
                                    name=f"ptr{kb}")
                     nc.tensor.transpose(
                         ptr[:], an[:, kb * 128:(kb + 1) * 128], identity16[:])
                     a_kb = p_sm.tile([128, HH], dt.float16, tag=f"aT{kb}")
                     nc.vector.tensor_copy(a_kb[:], ptr[:])
                     aT.append(a_kb)

                 def gate_mm(widx, uidx, rhs_u, rhs_u_sl, func, bias_nm):
                     Wq, Uq = gw_res[widx], gw_res[uidx]
                     outs = []
                     for f in range(KT):
                         pg = p_g.tile([128, HH], dt.float32, tag="gg",
                                       name=f"g{bias_nm}{f}")
                         if "gru" in ablate:
                             nc.tensor.matmul(pg[:], aT[0][:, 0:128], aT[0][:],
                                              start=True, stop=True)
                         else:
                             for k in range(KT):
                                 nc.tensor.matmul(
                                     pg[:], Uq[:, k, f * 128:(f + 1) * 128],
                                     rhs_u[k][:, rhs_u_sl] if rhs_u_sl else
                                     rhs_u[k][:],
                                     start=(k == 0), stop=False)
                             for k in range(KT):
                                 nc.tensor.matmul(
                                     pg[:], Wq[:, k, f * 128:(f + 1) * 128],
                                     aT[k][:],
                                     start=False, stop=(k == KT - 1))
                         og = p_sm.tile([128, HH], dt.float32,
                                        tag=f"g{bias_nm}{f}")
                         nc.scalar.activation(og[:], pg[:], func,
                                              bias=bias_tiles[(bias_nm, f)][:])
                         outs.append(og)
                     return outs

                 z_t = gate_mm(0, 1, hsh_prev, cs, SIG, "z")
                 r_t = gate_mm(2, 3, hsh_prev, cs, SIG, "r")
                 rh = []
                 for k in range(KT):
                     rhk = p_sm.tile([128, HH], dt.float16, tag=f"rh{k}")
                     nc.vector.tensor_mul(rhk[:], r_t[k][:], h32_prev[k][:, cs])
                     rh.append(rhk)
                 ht_t = gate_mm(4, 5, rh, None, TANH, "h")

                 # h' = h + z * (ht - h) on columns of this half
                 if not last:
                     ag_in = dram.tile([D, HH], dt.float16, tag=f"ag_in{X}",
                                       name=f"ag_in{X}")
                 for k in range(KT):
                     s1 = p_sm.tile([128, HH], dt.float32, tag="gsA")
                     nc.vector.tensor_sub(s1[:], ht_t[k][:], h32_prev[k][:, cs])
                     s2 = p_sm.tile([128, HH], dt.float32, tag="gsB")
                     nc.vector.tensor_mul(s2[:], z_t[k][:], s1[:])
                     nc.vector.tensor_add(h32_new[k][:, cs], h32_prev[k][:, cs],
                                          s2[:])
                     if last:
                         nc.sync.dma_start(out_p[k * 128:(k + 1) * 128, cs],
                                           h32_new[k][:, cs])
                     else:
                         nc.vector.tensor_copy(hsh_new[k][:, cs],
                                               h32_new[k][:, cs])
                         nc.sync.dma_start(ag_in[k * 128:(k + 1) * 128, :],
                                           hsh_new[k][:, cs])

                 if not last:
                     ag_out = dram.tile([NC_CORES * D, HH], dt.float16,
                                        tag=f"ag_out{X}", name=f"ag_out{X}",
                                        addr_space="Shared")
                     if "cc" in ablate or "ag" in ablate:
                         nc.sync.dma_start(ag_out[0:D, :], ag_in[:])
                     else:
                         nc.gpsimd.collective_compute(
                             "AllGather", mybir.AluOpType.bypass,
                             replica_groups=RG,
                             ins=[ag_in[:]], outs=[ag_out[:]])
                     ag_new[X] = ag_out

             if not last:
                 ag_prev = ag_new
                 hsh_prev, h32_prev = hsh_new, h32_new

    nc.finalize()
    return nc


_BUILT = None
TRACE = False
LAST_RESULT = None


_BUILT_R = {}


def _get_built(repeats=1, ablate=()):
    global _BUILT
    key = (repeats, tuple(ablate))
    if key != (1, ()):
        if key not in _BUILT_R:
            _BUILT_R[key] = build(repeats, ablate)
        return _BUILT_R[key]
    if _BUILT is None:
        _BUILT = build()
    return _BUILT


def prepare_in_maps(adjacency, annotations, W_prop, b_prop, Wz, Uz, bz,
                    Wr, Ur, br, Wh, Uh, bh):
    A = np.asarray(adjacency, np.float32)
    ann = np.asarray(annotations, np.float32)
    W_prop = np.asarray(W_prop, np.float32)
    b_prop = np.asarray(b_prop, np.float32)
    gw_all = np.stack([np.asarray(x, np.float32)
                       for x in (Wz, Uz, Wr, Ur, Wh, Uh)]).astype(np.float16)
    bz = np.asarray(bz, np.float32).reshape(D, 1)
    br = np.asarray(br, np.float32).reshape(D, 1)
    bh = np.asarray(bh, np.float32).reshape(D, 1)

    h0 = np.zeros((N, D), np.float32)
    h0[:, :ann.shape[1]] = ann
    h0t = np.ascontiguousarray(h0.T)           # [D, N] fp32
    h0t_r = h0t.astype(np.float16)
    A_T = np.ascontiguousarray(A.T)            # [2E*N, N]

    # shard layout: core c owns node blocks {128c..128c+127, 1024+128c..+127}
    shard_cols = [np.r_[128 * c:128 * c + 128, 1024 + 128 * c:1024 + 128 * c + 128]
                  for c in range(NC_CORES)]
    h0t_ag = np.ascontiguousarray(np.concatenate(
        [h0t_r[:, shard_cols[c]] for c in range(NC_CORES)], axis=0))

    in_maps = []
    for c in range(NC_CORES):
        in_maps.append({
            "at": np.ascontiguousarray(
                A_T[c * N:(c + 1) * N, :]).astype(np.float16),
            "h0t": h0t_ag,
            "h0sr": np.ascontiguousarray(h0t_r[:, shard_cols[c]]),
            "h0s": np.ascontiguousarray(h0t[:, shard_cols[c]]),
            "wc": W_prop[c].astype(np.float16),
            "gw": gw_all,
            "bpc": np.ascontiguousarray(b_prop[c].reshape(1, D)),
            "bzc": bz, "brc": br, "bhc": bh,
        })

    return in_maps


def kernel(**inputs):
    from concourse.bass_utils import run_bass_kernel_spmd

    in_maps = prepare_in_maps(
        **{k: inputs[k] for k in ("adjacency", "annotations", "W_prop", "b_prop",
                                  "Wz", "Uz", "bz", "Wr", "Ur", "br",
                                  "Wh", "Uh", "bh")})
    nc = _get_built()
    res = run_bass_kernel_spmd(nc, in_maps, list(range(NC_CORES)), trace=TRACE)
    global LAST_RESULT
    LAST_RESULT = res
    h = np.empty((N, D), np.float32)
    for c in range(NC_CORES):
        sh = res.results[c]["out"].T           # [SH, D] rows in shard order
        h[128 * c:128 * c + 128] = sh[:128]
        h[1024 + 128 * c:1024 + 128 * c + 128] = sh[128:]
    return h


# revision 34
# speedup vs baseline: 1.7734x; 1.0265x over previous
"""GGNN (gated graph NN) message-passing kernel for 8 Trainium2 NeuronCores.

Sharding: edge-type sharding. Core c owns edge-type block c of the adjacency
matrix (columns c*N..(c+1)*N of the [N, 2E*N] adjacency, pre-transposed on the
host) plus node shard c for the GRU update.

Per step, on core c (node shard split in halves A = block {128c},
B = block {1024+128c} — the blocks the two ReduceScatters deliver):
  stage1: t_c = h @ W_prop[c]                       [N, D]
          (emitted as half-A m-tiles then half-B so each half only
           depends on its own AllGather from the previous step)
  stage2: partial_a_c = A_cT.T @ t_c                [N, D]  in 4 sub-groups
          of 4 node-tiles (4 PSUM banks), RS-A issued after sub-group 1,
          RS-B after sub-group 3 (RS-A hides under sub-groups 2-3)
  tail, per half X in {A, B}:
      transpose a_X -> aT_X [D, 128]
      GRU gates on half X (fp16 matmuls, free dim 128)
      h'_X elementwise
      AG-X: AllGather(h'_X^T) -> [8*D, 128]
  Half-A gates run while RS-B is in flight; AG-A runs while half-B gates
  compute; AG-B overlaps next step's stage1 half-A matmuls.

Numerics: all matmuls in float16 (full PE rate at any free size; adjacency
0/1 is exact in fp16, weights/states lose ~2^-11 relative); collective
payloads fp16; accumulation fp32 in PSUM; elementwise GRU update in fp32.
"""
import sys
if "/opt/trn_rl_repo" not in sys.path:
    sys.path.insert(0, "/opt/trn_rl_repo")

import numpy as np
import ml_dtypes

NC_CORES = 8
N = 2048          # nodes
D = 512           # state dim
ANN = 256         # annotation dim
STEPS = 5
SH = N // NC_CORES   # 256 nodes per shard
HH = SH // 2         # 128 nodes per half-shard
KT = D // 128        # 4
MT = N // 128        # 16


def build(repeats=1, ablate=()):
    import concourse.bacc as bacc
    import concourse.mybir as mybir
    import concourse.tile as tile
    from concourse.masks import make_identity

    dt = mybir.dt
    nc = bacc.Bacc()
    at_p = nc.declare_dram_parameter("at", [N, N], dt.float16, isOutput=False)
    h0t_p = nc.declare_dram_parameter("h0t", [NC_CORES * D, SH], dt.float16,
                                      isOutput=False)
    h0sr_p = nc.declare_dram_parameter("h0sr", [D, SH], dt.float16, isOutput=False)
    h0s_p = nc.declare_dram_parameter("h0s", [D, SH], dt.float32, isOutput=False)
    wc_p = nc.declare_dram_parameter("wc", [D, D], dt.float16, isOutput=False)
    gw_p = nc.declare_dram_parameter("gw", [6, D, D], dt.float16, isOutput=False)
    bpc_p = nc.declare_dram_parameter("bpc", [1, D], dt.float32, isOutput=False)
    bz_p = nc.declare_dram_parameter("bzc", [D, 1], dt.float32, isOutput=False)
    br_p = nc.declare_dram_parameter("brc", [D, 1], dt.float32, isOutput=False)
    bh_p = nc.declare_dram_parameter("bhc", [D, 1], dt.float32, isOutput=False)
    out_p = nc.declare_dram_parameter("out", [D, SH], dt.float32, isOutput=True)
    RG = [list(range(NC_CORES))]

    from contextlib import ExitStack
    with tile.TileContext(nc) as tc, ExitStack() as stk:
        res = stk.enter_context(tc.tile_pool(name="res", bufs=1))
        # PSUM: bank-granular (8 banks). stage1/stage2 chains use p_mm (5),
        # gate chains + transposes use p_g (3).
        p_mm = stk.enter_context(tc.tile_pool(name="pmm", bufs=5, space="PSUM"))
        p_g = stk.enter_context(tc.tile_pool(name="pg", bufs=3, space="PSUM"))
        p_hc = stk.enter_context(tc.tile_pool(name="phc", bufs=6))
        p_t = stk.enter_context(tc.tile_pool(name="pt", bufs=1))
        p_pp = stk.enter_context(tc.tile_pool(name="ppp", bufs=1))
        p_asb = stk.enter_context(tc.tile_pool(name="pasb", bufs=4))
        p_sm = stk.enter_context(tc.tile_pool(name="psm", bufs=2))
        p_h = stk.enter_context(tc.tile_pool(name="ph", bufs=2))
        dram = stk.enter_context(tc.tile_pool(name="dram", bufs=2, space="DRAM"))

        # ---- setup: constants, weights, adjacency ----
        identity = res.tile([128, 128], dt.float32, tag="identity")
        make_identity(nc, identity[:])
        identity16 = res.tile([128, 128], dt.float16, tag="identity16")
        nc.vector.tensor_copy(identity16[:], identity[:])
        ones = res.tile([1, 128], dt.float32, tag="ones")
        nc.vector.memset(ones[:], 1.0)
        bpc_t = res.tile([1, D], dt.float32, tag="bpc")
        nc.sync.dma_start(bpc_t[:], bpc_p[:])
        pb = p_mm.tile([128, D], dt.float32, tag="mm")
        nc.tensor.matmul(pb[:], ones[:], bpc_t[:], start=True, stop=True)
        bias_bcast = res.tile([128, D], dt.float32, tag="bias_bcast")
        nc.vector.tensor_copy(bias_bcast[:], pb[:])

        bias_tiles = {}
        for nm, par in (("z", bz_p), ("r", br_p), ("h", bh_p)):
            for f in range(KT):
                bt = res.tile([128, 1], dt.float32, tag=f"b{nm}{f}")
                nc.sync.dma_start(bt[:], par[f * 128:(f + 1) * 128, :])
                bias_tiles[(nm, f)] = bt

        wc_t = []
        for k in range(KT):
            w = res.tile([128, D], dt.float16, tag=f"wc{k}")
            nc.sync.dma_start(w[:], wc_p[k * 128:(k + 1) * 128, :])
            wc_t.append(w)

        at_t = []
        for m in range(MT):
            a = res.tile([128, N], dt.float16, tag=f"at{m}")
            nc.sync.dma_start(a[:], at_p[m * 128:(m + 1) * 128, :])
            at_t.append(a)

        # resident GRU weights (fp16), loaded once
        gw_res = []
        for g in range(6):
            w = res.tile([128, KT, D], dt.float16, tag=f"gwr{g}")
            nc.scalar.dma_start(w[:], gw_p[g].rearrange("(k p) f -> p k f", p=128))
            gw_res.append(w)

        import concourse.mybir as _mb
        SIG = _mb.ActivationFunctionType.Sigmoid
        TANH = _mb.ActivationFunctionType.Tanh

        for rep in range(repeats):
          # step-0 h state
          hsh_prev = []   # h^T shard, fp16 (GRU U-term rhs)
          h32_prev = []   # h^T shard, fp32 (elementwise state)
          for k in range(KT):
            hr = p_h.tile([128, SH], dt.float16, tag=f"hnr{k}")
            nc.sync.dma_start(hr[:], h0sr_p[k * 128:(k + 1) * 128, :])
            hsh_prev.append(hr)
            h3 = p_h.tile([128, SH], dt.float32, tag=f"h32{k}")
            nc.sync.dma_start(h3[:], h0s_p[k * 128:(k + 1) * 128, :])
            h32_prev.append(h3)

          ag_prev = None   # pair (agA, agB) of [NC*D, HH] fp16

          for s in range(STEPS):
             # ---- pre-stage z/r U-terms (depend only on local h'^T, so
             # they fill the PE idle window while AG-A/AG-B are in flight;
             # consumed via a PSUM-preload identity matmul in gate_mm) ----
             uP = {}
             if "gru" not in ablate:
                 for gi, uidx in (("z", 1), ("r", 3)):
                     Uq = gw_res[uidx]
                     for X in range(2):
                         for f in range(KT):
                             pg = p_g.tile([128, HH], dt.float32, tag="gg",
                                           name=f"pu{gi}{X}{f}")
                             for k in range(KT):
                                 nc.tensor.matmul(
                                     pg[:], Uq[:, k, f * 128:(f + 1) * 128],
                                     hsh_prev[k][:, X * HH:(X + 1) * HH],
                                     start=(k == 0), stop=(k == KT - 1))
                             up = p_sm.tile([128, HH], dt.float16,
                                            tag=f"uP{gi}{X}{f}",
                                            name=f"uP{gi}{X}{f}")
                             if f % 2 == 0:
                                 nc.scalar.copy(up[:], pg[:])
                             else:
                                 nc.vector.tensor_copy(up[:], pg[:])
                             uP[(gi, X, f)] = up
             # ---- stage 1 + stage 2, software-pipelined ----
             # stage1 half-X m-tiles (mloc=X) only need AG-X of the previous
             # step. stage2 is split into two contraction phases: phase1
             # (m=0..7, needs only stage1-A) runs between stage1-A and
             # stage1-B so it overlaps the in-flight AG-B; phase2 (m=8..15)
             # adds the SBUF-staged phase-1 partials on the way out. This
             # moves RS-A's inputs ~15us earlier.
             t_tiles = [None] * MT

             def stage1(mloc):
                 for mp in range(8):
                     m = mp + 8 * mloc
                     if "s1" in ablate:
                         pt = p_mm.tile([128, D], dt.float32, tag="mm",
                                        name="pt")
                         nc.tensor.matmul(pt[:], wc_t[0][:, 0:128], wc_t[1][:],
                                          start=True, stop=True)
                     else:
                         hc = p_hc.tile([128, KT, 128], dt.float16, tag="hc",
                                        name="hc")
                         if s == 0:
                             blk = h0t_p[512 * mp:512 * (mp + 1),
                                         mloc * HH:(mloc + 1) * HH]
                         else:
                             blk = ag_prev[mloc][512 * mp:512 * (mp + 1), :]
                         nc.sync.dma_start(
                             hc[:], blk.rearrange("(k p) j -> p k j", p=128))
                         pt = p_mm.tile([128, D], dt.float32, tag="mm",
                                        name="pt")
                         for k in range(KT):
                             nc.tensor.matmul(pt[:], hc[:, k, :], wc_t[k][:],
                                              start=(k == 0), stop=(k == KT - 1))
                     tm = p_t.tile([128, D], dt.float16, tag=f"t{m}", name="tm")
                     nc.vector.tensor_add(tm[:], pt[:], bias_bcast[:])
                     t_tiles[m] = tm

             rs_ins = [dram.tile([N // 2, D], dt.float16, tag=f"rs_in{g}",
                                 name=f"rs_in{g}") for g in range(2)]
             rs_outs = []
             pp = {}

             def s2_phase1(sub):
                 if "s2" in ablate:
                     return
                 pas = [p_mm.tile([128, D], dt.float32, tag="mm",
                                  name=f"pa1_{sub}_{i}") for i in range(4)]
                 for m in range(8):
                     for i in range(4):
                         col = sub * 512 + i * 128
                         nc.tensor.matmul(pas[i][:],
                                          at_t[m][:, col:col + 128],
                                          t_tiles[m][:],
                                          start=(m == 0), stop=(m == 7))
                 for i in range(4):
                     ppt = p_pp.tile([128, D], dt.float32, tag=f"pp{sub}{i}",
                                     name=f"pp{sub}{i}")
                     if i % 2 == 0:
                         nc.scalar.copy(ppt[:], pas[i][:])
                     else:
                         nc.vector.tensor_copy(ppt[:], pas[i][:])
                     pp[(sub, i)] = ppt

             def s2_phase2(sub):
                 pas = [p_mm.tile([128, D], dt.float32, tag="mm",
                                  name=f"pa2_{sub}_{i}") for i in range(4)]
                 if "s2" in ablate:
                     for i in range(4):
                         nc.tensor.matmul(pas[i][:], t_tiles[0][:, 0:128],
                                          t_tiles[1][:], start=True, stop=True)
                 else:
                  for m in range(8, MT):
                     for i in range(4):
                         col = sub * 512 + i * 128
                         nc.tensor.matmul(pas[i][:],
                                          at_t[m][:, col:col + 128],
                                          t_tiles[m][:],
                                          start=(m == 8), stop=(m == MT - 1))
                 for i in range(4):
                     asb = p_asb.tile([128, D], dt.float16, tag="asb",
                                      name="asb")
                     if "s2" in ablate:
                         nc.vector.tensor_copy(asb[:], pas[i][:])
                     else:
                         nc.vector.tensor_add(asb[:], pas[i][:],
                                              pp[(sub, i)][:])
                     eng = nc.sync if i % 2 == 0 else nc.scalar
                     row = 512 * (sub % 2) + i * 128
                     eng.dma_start(rs_ins[sub // 2][row:row + 128, :], asb[:])
                 if sub % 2 == 1:
                     grp = sub // 2
                     # RS of this half: core c receives node block grp*1024+128c
                     rs_out = dram.tile([HH, D], dt.float16, tag=f"rs_out{grp}",
                                        name=f"rs_out{grp}")
                     if "cc" in ablate or "rs" in ablate:
                         nc.sync.dma_start(rs_out[:], rs_ins[grp][0:HH, :])
                     else:
                         nc.gpsimd.collective_compute(
                             "ReduceScatter", mybir.AluOpType.add,
                             replica_groups=RG,
                             ins=[rs_ins[grp][:]], outs=[rs_out[:]])
                     rs_outs.append(rs_out)

             stage1(0)
             s2_phase1(0)
             s2_phase1(1)
             stage1(1)
             s2_phase2(0)
             s2_phase2(1)      # issues RS-A
             s2_phase1(2)
             s2_phase1(3)
             s2_phase2(2)
             s2_phase2(3)      # issues RS-B

             # ---- per-half tail: transpose, gates, h', AG ----
             last = (s == STEPS - 1)
             hsh_new, h32_new = [], []
             ag_new = [None, None]
             for k in range(KT):
                 if not last:
                     hr = p_h.tile([128, SH], dt.float16, tag=f"hnr{k}",
                                   name=f"hnr{k}")
                     hsh_new.append(hr)
                 h3 = p_h.tile([128, SH], dt.float32, tag=f"h32{k}",
                               name=f"h32{k}")
                 h32_new.append(h3)

             for X in range(2):
                 cs = slice(X * HH, (X + 1) * HH)
                 an = p_sm.tile([128, D], dt.float16, tag=f"an{X}")
                 nc.sync.dma_start(an[:], rs_outs[X][:])
                 aT = []
                 for kb in range(KT):
                     ptr = p_g.tile([128, 128], dt.float16, tag="gg",
                                    name=f"ptr{kb}")
                     nc.tensor.transpose(
                         ptr[:], an[:, kb * 128:(kb + 1) * 128], identity16[:])
                     a_kb = p_sm.tile([128, HH], dt.float16, tag=f"aT{kb}")
                     nc.vector.tensor_copy(a_kb[:], ptr[:])
                     aT.append(a_kb)

                 def gate_mm(widx, uidx, rhs_u, func, bias_nm, pre=False):
                     Wq, Uq = gw_res[widx], gw_res[uidx]
                     outs = []
                     for f in range(KT):
                         pg = p_g.tile([128, HH], dt.float32, tag="gg",
                                       name=f"g{bias_nm}{f}")
                         if "gru" in ablate:
                             nc.tensor.matmul(pg[:], aT[0][:, 0:128], aT[0][:],
                                              start=True, stop=True)
                         else:
                             if pre:
                                 # preload PSUM with the pre-staged U-term
                                 nc.tensor.matmul(
                                     pg[:], identity16[:],
                                     uP[(bias_nm, X, f)][:],
                                     start=True, stop=False)
                             else:
                                 for k in range(KT):
                                     nc.tensor.matmul(
                                         pg[:], Uq[:, k, f * 128:(f + 1) * 128],
                                         rhs_u[k][:],
                                         start=(k == 0), stop=False)
                             for k in range(KT):
                                 nc.tensor.matmul(
                                     pg[:], Wq[:, k, f * 128:(f + 1) * 128],
                                     aT[k][:],
                                     start=False, stop=(k == KT - 1))
                         og = p_sm.tile([128, HH], dt.float32,
                                        tag=f"g{bias_nm}{f}")
                         nc.scalar.activation(og[:], pg[:], func,
                                              bias=bias_tiles[(bias_nm, f)][:])
                         outs.append(og)
                     return outs

                 z_t = gate_mm(0, 1, None, SIG, "z", pre=True)
                 r_t = gate_mm(2, 3, None, SIG, "r", pre=True)
                 rh = []
                 for k in range(KT):
                     rhk = p_sm.tile([128, HH], dt.float16, tag=f"rh{k}")
                     nc.vector.tensor_mul(rhk[:], r_t[k][:], h32_prev[k][:, cs])
                     rh.append(rhk)
                 ht_t = gate_mm(4, 5, rh, TANH, "h")

                 # h' = h + z * (ht - h) on columns of this half
                 if not last:
                     ag_in = dram.tile([D, HH], dt.float16, tag=f"ag_in{X}",
                                       name=f"ag_in{X}")
                 for k in range(KT):
                     s1 = p_sm.tile([128, HH], dt.float32, tag="gsA")
                     nc.vector.tensor_sub(s1[:], ht_t[k][:], h32_prev[k][:, cs])
                     s2 = p_sm.tile([128, HH], dt.float32, tag="gsB")
                     nc.vector.tensor_mul(s2[:], z_t[k][:], s1[:])
                     nc.vector.tensor_add(h32_new[k][:, cs], h32_prev[k][:, cs],
                                          s2[:])
                     if last:
                         nc.sync.dma_start(out_p[k * 128:(k + 1) * 128, cs],
                                           h32_new[k][:, cs])
                     else:
                         nc.vector.tensor_copy(hsh_new[k][:, cs],
                                               h32_new[k][:, cs])
                         nc.sync.dma_start(ag_in[k * 128:(k + 1) * 128, :],
                                           hsh_new[k][:, cs])

                 if not last:
                     ag_out = dram.tile([NC_CORES * D, HH], dt.float16,
                                        tag=f"ag_out{X}", name=f"ag_out{X}",
                                        addr_space="Shared")
                     if "cc" in ablate or "ag" in ablate:
                         nc.sync.dma_start(ag_out[0:D, :], ag_in[:])
                     else:
                         nc.gpsimd.collective_compute(
                             "AllGather", mybir.AluOpType.bypass,
                             replica_groups=RG,
                             ins=[ag_in[:]], outs=[ag_out[:]])
                     ag_new[X] = ag_out

             if not last:
                 ag_prev = ag_new
                 hsh_prev, h32_prev = hsh_new, h32_new

    nc.finalize()
    return nc


_BUILT = None
TRACE = False
LAST_RESULT = None


_BUILT_R = {}


def _get_built(repeats=1, ablate=()):
    global _BUILT
    key = (repeats, tuple(ablate))
    if key != (1, ()):
        if key not in _BUILT_R:
            _BUILT_R[key] = build(repeats, ablate)
        return _BUILT_R[key]
    if _BUILT is None:
        _BUILT = build()
    return _BUILT


def prepare_in_maps(adjacency, annotations, W_prop, b_prop, Wz, Uz, bz,
                    Wr, Ur, br, Wh, Uh, bh):
    A = np.asarray(adjacency, np.float32)
    ann = np.asarray(annotations, np.float32)
    W_prop = np.asarray(W_prop, np.float32)
    b_prop = np.asarray(b_prop, np.float32)
    gw_all = np.stack([np.asarray(x, np.float32)
                       for x in (Wz, Uz, Wr, Ur, Wh, Uh)]).astype(np.float16)
    bz = np.asarray(bz, np.float32).reshape(D, 1)
    br = np.asarray(br, np.float32).reshape(D, 1)
    bh = np.asarray(bh, np.float32).reshape(D, 1)

    h0 = np.zeros((N, D), np.float32)
    h0[:, :ann.shape[1]] = ann
    h0t = np.ascontiguousarray(h0.T)           # [D, N] fp32
    h0t_r = h0t.astype(np.float16)
    A_T = np.ascontiguousarray(A.T)            # [2E*N, N]

    # shard layout: core c owns node blocks {128c..128c+127, 1024+128c..+127}
    shard_cols = [np.r_[128 * c:128 * c + 128, 1024 + 128 * c:1024 + 128 * c + 128]
                  for c in range(NC_CORES)]
    h0t_ag = np.ascontiguousarray(np.concatenate(
        [h0t_r[:, shard_cols[c]] for c in range(NC_CORES)], axis=0))

    in_maps = []
    for c in range(NC_CORES):
        in_maps.append({
            "at": np.ascontiguousarray(
                A_T[c * N:(c + 1) * N, :]).astype(np.float16),
            "h0t": h0t_ag,
            "h0sr": np.ascontiguousarray(h0t_r[:, shard_cols[c]]),
            "h0s": np.ascontiguousarray(h0t[:, shard_cols[c]]),
            "wc": W_prop[c].astype(np.float16),
            "gw": gw_all,
            "bpc": np.ascontiguousarray(b_prop[c].reshape(1, D)),
            "bzc": bz, "brc": br, "bhc": bh,
        })

    return in_maps


def kernel(**inputs):
    from concourse.bass_utils import run_bass_kernel_spmd

    in_maps = prepare_in_maps(
        **{k: inputs[k] for k in ("adjacency", "annotations", "W_prop", "b_prop",
                                  "Wz", "Uz", "bz", "Wr", "Ur", "br",
                                  "Wh", "Uh", "bh")})
    nc = _get_built()
    res = run_bass_kernel_spmd(nc, in_maps, list(range(NC_CORES)), trace=TRACE)
    global LAST_RESULT
    LAST_RESULT = res
    h = np.empty((N, D), np.float32)
    for c in range(NC_CORES):
        sh = res.results[c]["out"].T           # [SH, D] rows in shard order
        h[128 * c:128 * c + 128] = sh[:128]
        h[1024 + 128 * c:1024 + 128 * c + 128] = sh[128:]
    return h
